# revision 1
# baseline (speedup 1.0000x reference)
"""Multi-head causal self-attention (B=2, S=2048, E=1024, H=16) on 8 TRN2 NeuronCores.

Sharding: tensor-parallel over heads (2 heads/core, both batches). Per core:
  - QKV projection for its 2 heads (q^T/k^T transposed layout, v natural)
  - causal flash-style attention, scores computed transposed (k on partitions)
    so no probability-matrix transposes are needed; softmax denominator comes
    from a ones-column appended to V
  - PE-transpose of the attention output, 8-way AllToAll to reshard from
    head-columns to token-rows, then row-parallel output projection.
Host side only reshapes/slices inputs and concatenates the 8 disjoint row
shards of the output.

Emission order pipelines batch 1's QKV under batch 0's (ACT-bound) attention.
"""

import numpy as np
import ml_dtypes

P = 128
B, S, E, H, D = 2, 2048, 1024, 16, 64
NCORES = 8
EB = E // P            # 8 e-blocks
BS = B * S             # 4096 flattened rows
SBB = S // P           # 16 s-blocks per batch
SB = BS // P           # 32 s-blocks global
HPC = H // NCORES      # 2 heads per core
CH = BS // NCORES      # 512 rows owned per core

_bf16 = ml_dtypes.bfloat16
_cache = {}


def _build(no_cc=False):
    from contextlib import ExitStack

    import concourse.tile as tile
    from concourse import bacc, mybir
    from concourse.masks import make_identity

    bf16 = mybir.dt.bfloat16
    f32 = mybir.dt.float32

    nc = bacc.Bacc("TRN2", target_bir_lowering=False, debug=False,
                   num_devices=NCORES)

    # host-side layouts are [partition, eblock, col] so each load is one DMA
    xT_d = nc.dram_tensor("xT", [P, EB, BS], bf16, kind="ExternalInput")
    wqk_d = nc.dram_tensor("wqk", [P, EB, 2 * P], bf16, kind="ExternalInput")
    wv_d = nc.dram_tensor("wv", [P, EB, P], bf16, kind="ExternalInput")
    wo_d = nc.dram_tensor("wo", [P, EB, E], bf16, kind="ExternalInput")
    bqk_d = nc.dram_tensor("bqk", [P, 2], f32, kind="ExternalInput")
    bv_d = nc.dram_tensor("bv", [1, P], bf16, kind="ExternalInput")
    bo_d = nc.dram_tensor("bo", [1, E], bf16, kind="ExternalInput")
    tri_d = nc.dram_tensor("tri", [P, P], bf16, kind="ExternalInput")
    # rank r owns interleaved token blocks {r, r+8, r+16, r+24}: one AllToAll
    # per batch, so batch 0's A2A + output projection hide under batch 1's
    # attention. out row-block st <-> global block st*8 + rank.
    out_d = nc.dram_tensor("out", [4, P, E], f32, kind="ExternalOutput")
    a2a_in = [nc.dram_tensor(f"a2a_in{b}", [NCORES, P, 2 * P], bf16)
              for b in range(B)]
    a2a_out = [nc.dram_tensor(f"a2a_out{b}", [NCORES, P, 2 * P], bf16)
               for b in range(B)]

    with tile.TileContext(nc) as tc, ExitStack() as ctx:
        consts = ctx.enter_context(tc.tile_pool(name="consts", bufs=1))
        work = ctx.enter_context(tc.tile_pool(name="work", bufs=1))
        xpool = ctx.enter_context(tc.tile_pool(name="xstream", bufs=2))
        epool = ctx.enter_context(tc.tile_pool(name="expst", bufs=2))
        small = ctx.enter_context(tc.tile_pool(name="small", bufs=4))
        opool = ctx.enter_context(tc.tile_pool(name="osb", bufs=2))
        pbig = ctx.enter_context(tc.tile_pool(name="pbig", bufs=2, space="PSUM"))
        ppv = ctx.enter_context(tc.tile_pool(name="ppv", bufs=2, space="PSUM"))
        psm = ctx.enter_context(tc.tile_pool(name="psm", bufs=2, space="PSUM"))

        wqk = consts.tile([P, EB, 2 * P], bf16, tag="wqk")
        wv = consts.tile([P, EB, P], bf16, tag="wv")
        wo = consts.tile([P, EB, E], bf16, tag="wo")
        bqk = consts.tile([P, 2], f32, tag="bqk")
        bv = consts.tile([1, P], bf16, tag="bv")
        bo = consts.tile([1, E], bf16, tag="bo")
        tri = consts.tile([P, P], bf16, tag="tri")
        ones1 = consts.tile([1, P], bf16, tag="ones1")
        ident = consts.tile([P, P], bf16, tag="ident")

        nc.sync.dma_start(wqk[:], wqk_d[:, :, :])
        nc.sync.dma_start(wv[:], wv_d[:, :, :])
        nc.sync.dma_start(bqk[:], bqk_d[:, :])
        nc.sync.dma_start(bv[:1, :], bv_d[:, :])
        nc.sync.dma_start(tri[:], tri_d[:, :])
        nc.vector.memset(ones1[:1, :], 1.0)
        make_identity(nc, ident[:])

        qkT = [work.tile([P, 2, S], bf16, tag=f"qkT{b}", name=f"qkT{b}")
               for b in range(B)]
        vsb = [work.tile([P, SBB, HPC, 66], bf16, tag=f"vsb{b}", name=f"vsb{b}")
               for b in range(B)]
        attn = [work.tile([P, SBB, HPC * D], bf16, tag=f"attn{b}", name=f"attn{b}")
                for b in range(B)]
        attnT = [work.tile([P, S], bf16, tag=f"attnT{b}", name=f"attnT{b}")
                 for b in range(B)]

        def qkv_pieces(b):
            """QKV projection for batch b, one 512-token chunk per piece.

            Chunks are emitted suffix-first: causal score block kb only needs
            token columns >= kb*128, so late chunks unblock the small k-blocks
            early and ACT (exp) can start before the whole projection is done.
            """
            nc.vector.memset(vsb[b][:], 1.0)
            for sc in reversed(range(S // 512)):
                gc = b * S + sc * 512  # global col
                xc = xpool.tile([P, EB, 512], bf16, tag="xc", name="xc")
                nc.sync.dma_start(xc[:], xT_d[:, :, gc:gc + 512])
                for db in range(2):
                    ps = psm.tile([P, 512], f32, tag="mid", name="psqk")
                    for eb in range(EB):
                        nc.tensor.matmul(
                            ps[:],
                            lhsT=wqk[:, eb, db * P:(db + 1) * P],
                            rhs=xc[:, eb, :],
                            start=(eb == 0), stop=(eb == EB - 1),
                        )
                    nc.vector.tensor_scalar_add(
                        qkT[b][:, db, sc * 512:(sc + 1) * 512], ps[:],
                        bqk[:, db:db + 1])
                    yield
                for si in range(4):
                    sb = sc * 4 + si
                    pv_ = psm.tile([P, P], f32, tag="mid", name="psv")
                    for eb in range(EB):
                        nc.tensor.matmul(
                            pv_[:], lhsT=xc[:, eb, si * P:(si + 1) * P],
                            rhs=wv[:, eb, :], start=(eb == 0), stop=False)
                    nc.tensor.matmul(pv_[:], lhsT=ones1[:1, :], rhs=bv[:1, :],
                                     start=False, stop=True)
                    nc.vector.tensor_copy(vsb[b][:, sb, 0, 0:64], pv_[:, 0:64])
                    nc.vector.tensor_copy(vsb[b][:, sb, 1, 0:64], pv_[:, 64:128])
                    yield

        def score_pieces(b, h, expst):
            """scores^T + exp for one (batch, head), one k-block per piece.

            k-blocks run high-to-low to match qkv_pieces' suffix-first order.
            """
            hs = slice(h * 64, (h + 1) * 64)
            expst.extend([None] * SBB)
            for kb in reversed(range(SBB)):
                L = S - kb * P
                # high-kb tiles are small: an extra buffer lets the next
                # batch's (reversed) scores start while this batch's PV is
                # still reading the lower k-blocks
                et = epool.tile([P, L], bf16, tag=f"e{kb}", name=f"e{kb}",
                                bufs=4 if kb >= 8 else 2)
                off = kb * P
                pos = 0
                while pos < L:  # 1024-wide psum tiles: 1 exp op per tile
                    c = min(1024, L - pos)
                    ps = pbig.tile([P, 1024], f32, tag="big", name="pssc")
                    for c0 in range(0, c, 512):
                        w = min(512, c - c0)
                        nc.tensor.matmul(
                            ps[:, c0:c0 + w],
                            lhsT=qkT[b][hs, 1, off:off + P],
                            rhs=qkT[b][hs, 0, off + pos + c0:off + pos + c0 + w],
                            start=True, stop=True)
                    nc.scalar.activation(
                        et[:, pos:pos + c], ps[:, :c],
                        mybir.ActivationFunctionType.Exp)
                    pos += c
                # zero the invalid (q < k) half of the diagonal block
                nc.gpsimd.tensor_mul(et[:, 0:P], et[:, 0:P], tri[:])
                expst[kb] = et
                yield

        def pv_pieces(b, h, expst):
            """PV + normalize for one (batch, head), one q-tile per piece."""
            hs = slice(h * 64, (h + 1) * 64)
            for qt in range(SBB):
                pp = ppv.tile([P, 66], f32, tag="pv", name="pspv")
                for kb in range(qt + 1):
                    nc.tensor.matmul(
                        pp[:, 0:65],
                        lhsT=expst[kb][:, (qt - kb) * P:(qt - kb) * P + P],
                        rhs=vsb[b][:, kb, h, 0:65],
                        start=(kb == 0), stop=(kb == qt))
                rc = small.tile([P, 1], f32, tag="recip", name="rc")
                nc.vector.reciprocal(rc[:], pp[:, 64:65])
                nc.vector.tensor_scalar_mul(
                    attn[b][:, qt, hs], pp[:, 0:64], rc[:])
                yield

        def transpose_pieces(b):
            """attn [token, e] -> attnT [e, token] via PE transposes."""
            for blk in range(SBB):
                pt = psm.tile([P, P], bf16, tag="mid", name="pst")
                nc.tensor.transpose(pt[:], attn[b][:, blk, :], ident[:])
                nc.vector.tensor_copy(attnT[b][:, blk * P:(blk + 1) * P], pt[:])
                yield
            # two strided DMAs: chunk j of the bounce gets blocks {j, j+8}
            for t in range(2):
                nc.sync.dma_start(
                    a2a_in[b].ap().rearrange(
                        "j p (t c) -> p j t c", t=2)[:, :, t, :],
                    attnT[b][:, t * NCORES * P:(t + 1) * NCORES * P].rearrange(
                        "p (j c) -> p j c", c=P))
            yield

        def interleave(*gens):
            gens = list(gens)
            while gens:
                gens = [g for g in gens if next(g, StopIteration) is not StopIteration]

        def paced(qg, score_gens, pv_gens=()):
            """Weave one qkv stream with score/pv streams, pacing emission so
            every score k-block is emitted AFTER the qkv chunk that writes the
            qkT columns it reads (Tile only tracks writer->reader deps in
            emission order). qkv chunk g (suffix-first) unlocks score k-blocks
            [12-4g, 15-4g]."""
            for g in range(4):
                for _ in range(6):
                    next(qg, None)
                for _ in range(4):
                    for sg in score_gens:
                        next(sg, None)
                    for pg in pv_gens:
                        next(pg, None)
            interleave(qg, *score_gens, *pv_gens)

        atf = [work.tile([P, EB, 2 * P], bf16, tag=f"atf{b}", name=f"atf{b}")
               for b in range(B)]

        def a2a_batch(b):
            """AllToAll batch b: head-columns -> my two token blocks."""
            if no_cc:
                # cost-model variant: TimelineSim can't simulate collectives
                for j in range(NCORES):
                    nc.sync.dma_start(a2a_out[b][j], a2a_in[b][j])
            else:
                nc.gpsimd.collective_compute(
                    "AllToAll", mybir.AluOpType.bypass,
                    replica_groups=[list(range(NCORES))],
                    ins=[a2a_in[b].ap()], outs=[a2a_out[b].ap()])
            nc.sync.dma_start(
                atf[b][:, :, :],
                a2a_out[b].ap().rearrange("j p c -> p j c"))

        def oproj_batch(b):
            """Output projection of my two token blocks of batch b."""
            for st in range(2):
                ot = opool.tile([P, E], f32, tag="o", name="ot")
                po = pbig.tile([P, 1024], f32, tag="big", name="pso")
                for oh in range(2):
                    for eb in range(EB):
                        nc.tensor.matmul(
                            po[:, oh * 512:(oh + 1) * 512],
                            lhsT=atf[b][:, eb, st * P:(st + 1) * P],
                            rhs=wo[:, eb, oh * 512:(oh + 1) * 512],
                            start=(eb == 0), stop=False)
                    nc.tensor.matmul(po[:, oh * 512:(oh + 1) * 512],
                                     lhsT=ones1[:1, :],
                                     rhs=bo[:1, oh * 512:(oh + 1) * 512],
                                     start=False, stop=True)
                nc.vector.tensor_copy(ot[:], po[:])
                nc.sync.dma_start(out_d[b * 2 + st], ot[:])

        # ---- pipelined emission (priorities; Tile schedules by readiness) ----
        e00, e01, e10, e11 = [], [], [], []
        paced(qkv_pieces(0),
              [score_pieces(0, 0, e00), score_pieces(0, 1, e01)])
        paced(qkv_pieces(1),
              [score_pieces(1, 0, e10), score_pieces(1, 1, e11)],
              [pv_pieces(0, 0, e00), pv_pieces(0, 1, e01)])
        nc.sync.dma_start(wo[:], wo_d[:, :, :])  # needed only at out-proj
        nc.sync.dma_start(bo[:1, :], bo_d[:, :])
        interleave(pv_pieces(1, 0, e10), pv_pieces(1, 1, e11),
                   transpose_pieces(0))
        a2a_batch(0)
        oproj_batch(0)          # hides under batch-1 attention tail
        interleave(transpose_pieces(1))
        a2a_batch(1)
        oproj_batch(1)

    nc.compile()
    return nc


def _in_maps(x, W_qkv, b_qkv, W_o, b_o):
    # [partition, eblock, col] layouts (see dram tensor decls)
    xT = np.ascontiguousarray(
        x.reshape(BS, EB, P).transpose(2, 1, 0)).astype(_bf16)
    wo = np.ascontiguousarray(
        W_o.reshape(EB, P, E).transpose(1, 0, 2)).astype(_bf16)
    bo = np.asarray(b_o).reshape(1, E).astype(_bf16)
    tri = np.triu(np.ones((P, P), np.float32)).astype(_bf16)
    maps = []
    for c in range(NCORES):
        o = c * HPC * D
        q_sl = slice(o, o + HPC * D)
        k_sl = slice(E + o, E + o + HPC * D)
        v_sl = slice(2 * E + o, 2 * E + o + HPC * D)
        wqk = np.concatenate(
            [W_qkv[:, q_sl] * 0.125, W_qkv[:, k_sl]], axis=1)
        maps.append({
            "xT": xT,
            "wqk": np.ascontiguousarray(
                wqk.reshape(EB, P, 2 * P).transpose(1, 0, 2)).astype(_bf16),
            "wv": np.ascontiguousarray(
                W_qkv[:, v_sl].reshape(EB, P, P).transpose(1, 0, 2)).astype(_bf16),
            "wo": wo,
            "bqk": np.stack([b_qkv[q_sl] * 0.125,
                             b_qkv[k_sl]], axis=1).astype(np.float32),
            "bv": b_qkv[v_sl].reshape(1, P).astype(_bf16),
            "bo": bo,
            "tri": tri,
        })
    return maps


def kernel(x, W_qkv, b_qkv, W_o, b_o, mask):
    from concourse.bass_utils import run_bass_kernel_spmd

    if "nc" not in _cache:
        _cache["nc"] = _build()
    nc = _cache["nc"]
    maps = _in_maps(np.asarray(x, np.float32), np.asarray(W_qkv, np.float32),
                    np.asarray(b_qkv, np.float32), np.asarray(W_o, np.float32),
                    np.asarray(b_o, np.float32))
    res = run_bass_kernel_spmd(nc, maps, list(range(NCORES)))
    # rank r's out[st] is global 128-token block st*8 + r
    full = np.empty((SB, P, E), np.float32)
    for r in range(NCORES):
        full[r::NCORES] = res.results[r]["out"]
    return full.reshape(B, S, E).astype(np.float32)



# revision 2
# speedup vs baseline: 3.6977x; 3.6977x over previous
"""Multi-head causal self-attention (B=2, S=2048, E=1024, H=16) on 8 TRN2 NeuronCores.

Sharding: tensor-parallel over heads (2 heads/core, both batches). Per core:
  - QKV projection for its 2 heads (q^T/k^T transposed layout, v natural)
  - causal flash-style attention, scores computed transposed (k on partitions)
    so no probability-matrix transposes are needed; softmax denominator comes
    from a ones-column appended to V
  - PE-transpose of the attention output, 8-way AllToAll to reshard from
    head-columns to token-rows, then row-parallel output projection.

Host<->device traffic is minimized: x arrives token-sharded (1/8 per core,
bf16) and is AllGathered on-device; W_o arrives row-sharded and is
AllGathered on-device (its latency hides under attention); the output leaves
as bf16 and is upcast on host. The jitted PJRT callable is built once and
reused, so steady-state calls pay no jax retrace.

Emission order pipelines batch 1's QKV under batch 0's (ACT-bound) attention.
"""

import numpy as np
import ml_dtypes

P = 128
B, S, E, H, D = 2, 2048, 1024, 16, 64
NCORES = 8
EB = E // P            # 8 e-blocks
BS = B * S             # 4096 flattened rows
SBB = S // P           # 16 s-blocks per batch
SB = BS // P           # 32 s-blocks global
HPC = H // NCORES      # 2 heads per core
CH = BS // NCORES      # 512 rows owned per core

_bf16 = ml_dtypes.bfloat16
_cache = {}


def _build(no_cc=False):
    from contextlib import ExitStack

    import concourse.tile as tile
    from concourse import bacc, mybir
    from concourse.masks import make_identity

    bf16 = mybir.dt.bfloat16
    f32 = mybir.dt.float32

    nc = bacc.Bacc("TRN2", target_bir_lowering=False, debug=False,
                   num_devices=NCORES)

    grp = [list(range(NCORES))]

    # host-side layouts are [partition, eblock, col] so each load is one DMA.
    # xin: this core's 512-token column slice of x^T (tokens [512r, 512r+512)).
    xin_d = nc.dram_tensor("xin", [P, EB, CH], bf16, kind="ExternalInput")
    wqk_d = nc.dram_tensor("wqk", [P, EB, 2 * P], bf16, kind="ExternalInput")
    wv_d = nc.dram_tensor("wv", [P, EB, P], bf16, kind="ExternalInput")
    # wos: this core's 128-row slice of W_o (rows [128r, 128r+128)).
    wos_d = nc.dram_tensor("wos", [P, E], bf16, kind="ExternalInput")
    bqk_d = nc.dram_tensor("bqk", [P, 2], f32, kind="ExternalInput")
    bv_d = nc.dram_tensor("bv", [1, P], bf16, kind="ExternalInput")
    bo_d = nc.dram_tensor("bo", [1, E], bf16, kind="ExternalInput")
    tri_d = nc.dram_tensor("tri", [P, P], bf16, kind="ExternalInput")
    # rank r owns interleaved token blocks {r, r+8, r+16, r+24}: one AllToAll
    # per batch, so batch 0's A2A + output projection hide under batch 1's
    # attention. out row-block st <-> global block st*8 + rank.
    out_d = nc.dram_tensor("out", [4, P, E], bf16, kind="ExternalOutput")
    # collective bounce buffers (collectives can't touch I/O tensors).
    xag_in = nc.dram_tensor("xag_in", [P, EB, CH], bf16)
    xag_out = nc.dram_tensor("xag_out", [NCORES * P, EB, CH], bf16)
    wag_in = nc.dram_tensor("wag_in", [P, E], bf16)
    wag_out = nc.dram_tensor("wag_out", [NCORES * P, E], bf16)
    a2a_in = [nc.dram_tensor(f"a2a_in{b}", [NCORES, P, 2 * P], bf16)
              for b in range(B)]
    a2a_out = [nc.dram_tensor(f"a2a_out{b}", [NCORES, P, 2 * P], bf16)
               for b in range(B)]

    def all_gather(ins, outs):
        if no_cc:
            # cost-model variant: TimelineSim can't simulate collectives
            n = ins.shape[0]
            for j in range(NCORES):
                nc.sync.dma_start(outs[j * n:(j + 1) * n], ins[:])
        else:
            nc.gpsimd.collective_compute(
                "AllGather", mybir.AluOpType.bypass, replica_groups=grp,
                ins=[ins.ap()], outs=[outs.ap()])

    with tile.TileContext(nc) as tc, ExitStack() as ctx:
        consts = ctx.enter_context(tc.tile_pool(name="consts", bufs=1))
        work = ctx.enter_context(tc.tile_pool(name="work", bufs=1))
        xpool = ctx.enter_context(tc.tile_pool(name="xstream", bufs=2))
        epool = ctx.enter_context(tc.tile_pool(name="expst", bufs=2))
        small = ctx.enter_context(tc.tile_pool(name="small", bufs=4))
        opool = ctx.enter_context(tc.tile_pool(name="osb", bufs=2))
        pbig = ctx.enter_context(tc.tile_pool(name="pbig", bufs=2, space="PSUM"))
        ppv = ctx.enter_context(tc.tile_pool(name="ppv", bufs=2, space="PSUM"))
        psm = ctx.enter_context(tc.tile_pool(name="psm", bufs=2, space="PSUM"))

        # x AllGather first — everything downstream waits on it.
        nc.sync.dma_start(xag_in[:, :, :], xin_d[:, :, :])
        all_gather(xag_in, xag_out)
        # W_o AllGather — only needed at output projection, hides completely.
        nc.sync.dma_start(wag_in[:, :], wos_d[:, :])
        all_gather(wag_in, wag_out)

        wqk = consts.tile([P, EB, 2 * P], bf16, tag="wqk")
        wv = consts.tile([P, EB, P], bf16, tag="wv")
        wo = consts.tile([P, EB, E], bf16, tag="wo")
        bqk = consts.tile([P, 2], f32, tag="bqk")
        bv = consts.tile([1, P], bf16, tag="bv")
        bo = consts.tile([1, E], bf16, tag="bo")
        tri = consts.tile([P, P], bf16, tag="tri")
        ones1 = consts.tile([1, P], bf16, tag="ones1")
        ident = consts.tile([P, P], bf16, tag="ident")

        nc.sync.dma_start(wqk[:], wqk_d[:, :, :])
        nc.sync.dma_start(wv[:], wv_d[:, :, :])
        nc.sync.dma_start(bqk[:], bqk_d[:, :])
        nc.sync.dma_start(bv[:1, :], bv_d[:, :])
        nc.sync.dma_start(tri[:], tri_d[:, :])
        nc.vector.memset(ones1[:1, :], 1.0)
        make_identity(nc, ident[:])

        qkT = [work.tile([P, 2, S], bf16, tag=f"qkT{b}", name=f"qkT{b}")
               for b in range(B)]
        vsb = [work.tile([P, SBB, HPC, 66], bf16, tag=f"vsb{b}", name=f"vsb{b}")
               for b in range(B)]
        attn = [work.tile([P, SBB, HPC * D], bf16, tag=f"attn{b}", name=f"attn{b}")
                for b in range(B)]
        attnT = [work.tile([P, S], bf16, tag=f"attnT{b}", name=f"attnT{b}")
                 for b in range(B)]

        def qkv_pieces(b):
            """QKV projection for batch b, one 512-token chunk per piece.

            Chunks are emitted suffix-first: causal score block kb only needs
            token columns >= kb*128, so late chunks unblock the small k-blocks
            early and ACT (exp) can start before the whole projection is done.
            """
            nc.vector.memset(vsb[b][:], 1.0)
            for sc in reversed(range(S // CH)):
                g = b * (S // CH) + sc   # global 512-token chunk = rank g's shard
                xc = xpool.tile([P, EB, CH], bf16, tag="xc", name="xc")
                nc.sync.dma_start(xc[:], xag_out[g * P:(g + 1) * P, :, :])
                for db in range(2):
                    ps = psm.tile([P, CH], f32, tag="mid", name="psqk")
                    for eb in range(EB):
                        nc.tensor.matmul(
                            ps[:],
                            lhsT=wqk[:, eb, db * P:(db + 1) * P],
                            rhs=xc[:, eb, :],
                            start=(eb == 0), stop=(eb == EB - 1),
                        )
                    nc.vector.tensor_scalar_add(
                        qkT[b][:, db, sc * CH:(sc + 1) * CH], ps[:],
                        bqk[:, db:db + 1])
                    yield
                for si in range(4):
                    sb = sc * 4 + si
                    pv_ = psm.tile([P, P], f32, tag="mid", name="psv")
                    for eb in range(EB):
                        nc.tensor.matmul(
                            pv_[:], lhsT=xc[:, eb, si * P:(si + 1) * P],
                            rhs=wv[:, eb, :], start=(eb == 0), stop=False)
                    nc.tensor.matmul(pv_[:], lhsT=ones1[:1, :], rhs=bv[:1, :],
                                     start=False, stop=True)
                    nc.vector.tensor_copy(vsb[b][:, sb, 0, 0:64], pv_[:, 0:64])
                    nc.vector.tensor_copy(vsb[b][:, sb, 1, 0:64], pv_[:, 64:128])
                    yield

        def score_pieces(b, h, expst):
            """scores^T + exp for one (batch, head), one k-block per piece.

            k-blocks run high-to-low to match qkv_pieces' suffix-first order.
            """
            hs = slice(h * 64, (h + 1) * 64)
            expst.extend([None] * SBB)
            for kb in reversed(range(SBB)):
                L = S - kb * P
                # high-kb tiles are small: an extra buffer lets the next
                # batch's (reversed) scores start while this batch's PV is
                # still reading the lower k-blocks
                et = epool.tile([P, L], bf16, tag=f"e{kb}", name=f"e{kb}",
                                bufs=4 if kb >= 8 else 2)
                off = kb * P
                pos = 0
                while pos < L:  # 1024-wide psum tiles: 1 exp op per tile
                    c = min(1024, L - pos)
                    ps = pbig.tile([P, 1024], f32, tag="big", name="pssc")
                    for c0 in range(0, c, 512):
                        w = min(512, c - c0)
                        nc.tensor.matmul(
                            ps[:, c0:c0 + w],
                            lhsT=qkT[b][hs, 1, off:off + P],
                            rhs=qkT[b][hs, 0, off + pos + c0:off + pos + c0 + w],
                            start=True, stop=True)
                    nc.scalar.activation(
                        et[:, pos:pos + c], ps[:, :c],
                        mybir.ActivationFunctionType.Exp)
                    pos += c
                # zero the invalid (q < k) half of the diagonal block
                nc.gpsimd.tensor_mul(et[:, 0:P], et[:, 0:P], tri[:])
                expst[kb] = et
                yield

        def pv_pieces(b, h, expst):
            """PV + normalize for one (batch, head), one q-tile per piece."""
            hs = slice(h * 64, (h + 1) * 64)
            for qt in range(SBB):
                pp = ppv.tile([P, 66], f32, tag="pv", name="pspv")
                for kb in range(qt + 1):
                    nc.tensor.matmul(
                        pp[:, 0:65],
                        lhsT=expst[kb][:, (qt - kb) * P:(qt - kb) * P + P],
                        rhs=vsb[b][:, kb, h, 0:65],
                        start=(kb == 0), stop=(kb == qt))
                rc = small.tile([P, 1], f32, tag="recip", name="rc")
                nc.vector.reciprocal(rc[:], pp[:, 64:65])
                nc.vector.tensor_scalar_mul(
                    attn[b][:, qt, hs], pp[:, 0:64], rc[:])
                yield

        def transpose_pieces(b):
            """attn [token, e] -> attnT [e, token] via PE transposes."""
            for blk in range(SBB):
                pt = psm.tile([P, P], bf16, tag="mid", name="pst")
                nc.tensor.transpose(pt[:], attn[b][:, blk, :], ident[:])
                nc.vector.tensor_copy(attnT[b][:, blk * P:(blk + 1) * P], pt[:])
                yield
            # two strided DMAs: chunk j of the bounce gets blocks {j, j+8}
            for t in range(2):
                nc.sync.dma_start(
                    a2a_in[b].ap().rearrange(
                        "j p (t c) -> p j t c", t=2)[:, :, t, :],
                    attnT[b][:, t * NCORES * P:(t + 1) * NCORES * P].rearrange(
                        "p (j c) -> p j c", c=P))
            yield

        def interleave(*gens):
            gens = list(gens)
            while gens:
                gens = [g for g in gens if next(g, StopIteration) is not StopIteration]

        def paced(qg, score_gens, pv_gens=()):
            """Weave one qkv stream with score/pv streams, pacing emission so
            every score k-block is emitted AFTER the qkv chunk that writes the
            qkT columns it reads (Tile only tracks writer->reader deps in
            emission order). qkv chunk g (suffix-first) unlocks score k-blocks
            [12-4g, 15-4g]."""
            for g in range(4):
                for _ in range(6):
                    next(qg, None)
                for _ in range(4):
                    for sg in score_gens:
                        next(sg, None)
                    for pg in pv_gens:
                        next(pg, None)
            interleave(qg, *score_gens, *pv_gens)

        atf = [work.tile([P, EB, 2 * P], bf16, tag=f"atf{b}", name=f"atf{b}")
               for b in range(B)]

        def a2a_batch(b):
            """AllToAll batch b: head-columns -> my two token blocks."""
            if no_cc:
                # cost-model variant: TimelineSim can't simulate collectives
                for j in range(NCORES):
                    nc.sync.dma_start(a2a_out[b][j], a2a_in[b][j])
            else:
                nc.gpsimd.collective_compute(
                    "AllToAll", mybir.AluOpType.bypass,
                    replica_groups=grp,
                    ins=[a2a_in[b].ap()], outs=[a2a_out[b].ap()])
            nc.sync.dma_start(
                atf[b][:, :, :],
                a2a_out[b].ap().rearrange("j p c -> p j c"))

        def oproj_batch(b):
            """Output projection of my two token blocks of batch b."""
            for st in range(2):
                ot = opool.tile([P, E], bf16, tag="o", name="ot")
                po = pbig.tile([P, 1024], f32, tag="big", name="pso")
                for oh in range(2):
                    for eb in range(EB):
                        nc.tensor.matmul(
                            po[:, oh * 512:(oh + 1) * 512],
                            lhsT=atf[b][:, eb, st * P:(st + 1) * P],
                            rhs=wo[:, eb, oh * 512:(oh + 1) * 512],
                            start=(eb == 0), stop=False)
                    nc.tensor.matmul(po[:, oh * 512:(oh + 1) * 512],
                                     lhsT=ones1[:1, :],
                                     rhs=bo[:1, oh * 512:(oh + 1) * 512],
                                     start=False, stop=True)
                nc.vector.tensor_copy(ot[:], po[:])
                nc.sync.dma_start(out_d[b * 2 + st], ot[:])

        # ---- pipelined emission (priorities; Tile schedules by readiness) ----
        e00, e01, e10, e11 = [], [], [], []
        paced(qkv_pieces(0),
              [score_pieces(0, 0, e00), score_pieces(0, 1, e01)])
        paced(qkv_pieces(1),
              [score_pieces(1, 0, e10), score_pieces(1, 1, e11)],
              [pv_pieces(0, 0, e00), pv_pieces(0, 1, e01)])
        # full W_o materializes from the (long-finished) AllGather
        nc.sync.dma_start(wo[:], wag_out.ap().rearrange("(e p) n -> p e n", p=P))
        nc.sync.dma_start(bo[:1, :], bo_d[:, :])
        interleave(pv_pieces(1, 0, e10), pv_pieces(1, 1, e11),
                   transpose_pieces(0))
        a2a_batch(0)
        oproj_batch(0)          # hides under batch-1 attention tail
        interleave(transpose_pieces(1))
        a2a_batch(1)
        oproj_batch(1)

    nc.compile()
    return nc


def _make_runner(nc):
    """Build the shard_map'd PJRT callable ONCE; reuse across calls.

    Mirrors concourse.bass2jax.run_bass_via_pjrt, which rebuilds the jit
    (full jax retrace) and re-zeros output donation buffers on every call.
    """
    import jax
    import jax.numpy as jnp  # noqa: F401
    from jax.sharding import Mesh, PartitionSpec
    from jax.experimental.shard_map import shard_map
    from concourse import bass2jax, mybir

    bass2jax.install_neuronx_cc_hook()
    assert nc.dbg_addr is None

    partition_name = nc.partition_id_tensor.name if nc.partition_id_tensor else None
    in_names, out_names, out_avals, zero_outs = [], [], [], []
    for alloc in nc.m.functions[0].allocations:
        if not isinstance(alloc, mybir.MemoryLocationSet):
            continue
        name = alloc.memorylocations[0].name
        if alloc.kind == "ExternalInput":
            if name != partition_name:
                in_names.append(name)
        elif alloc.kind == "ExternalOutput":
            out_names.append(name)
            shape = tuple(alloc.tensor_shape)
            dtype = mybir.dt.np(alloc.dtype)
            out_avals.append(jax.core.ShapedArray(shape, dtype))
            zero_outs.append(np.zeros(shape, dtype))
    n_params = len(in_names)
    n_outs = len(out_avals)
    all_names = in_names + out_names + ([partition_name] if partition_name else [])
    donate = tuple(range(n_params, n_params + n_outs))

    def _body(*args):
        operands = list(args)
        if partition_name is not None:
            operands.append(bass2jax.partition_id_tensor())
        outs = bass2jax._bass_exec_p.bind(
            *operands,
            out_avals=tuple(out_avals),
            in_names=tuple(all_names),
            out_names=tuple(out_names),
            lowering_input_output_aliases=(),
            sim_require_finite=True,
            sim_require_nnan=True,
            nc=nc,
        )
        return tuple(outs)

    devices = jax.devices()[:NCORES]
    mesh = Mesh(np.asarray(devices), ("core",))
    in_specs = (PartitionSpec("core"),) * (n_params + n_outs)
    out_specs = (PartitionSpec("core"),) * n_outs
    sharded = jax.jit(
        shard_map(_body, mesh=mesh, in_specs=in_specs, out_specs=out_specs,
                  check_rep=False),
        donate_argnums=donate, keep_unused=True)
    # donation consumes the *device* buffer; the host zeros are reusable.
    concat_zeros = [
        np.zeros((NCORES * z.shape[0], *z.shape[1:]), z.dtype) for z in zero_outs
    ]

    def run(maps):
        concat_in = [
            np.concatenate([np.asarray(m[name]) for m in maps], axis=0)
            for name in in_names
        ]
        out_arrs = sharded(*concat_in, *concat_zeros)
        return [
            {name: np.asarray(out_arrs[i]).reshape(NCORES, *out_avals[i].shape)[c]
             for i, name in enumerate(out_names)}
            for c in range(NCORES)
        ]

    return run


def _in_maps(x, W_qkv, b_qkv, W_o, b_o):
    # [partition, eblock, col] layouts (see dram tensor decls)
    xT = np.ascontiguousarray(
        x.reshape(BS, EB, P).transpose(2, 1, 0)).astype(_bf16)
    bo = np.asarray(b_o).reshape(1, E).astype(_bf16)
    tri = np.triu(np.ones((P, P), np.float32)).astype(_bf16)
    maps = []
    for c in range(NCORES):
        o = c * HPC * D
        q_sl = slice(o, o + HPC * D)
        k_sl = slice(E + o, E + o + HPC * D)
        v_sl = slice(2 * E + o, 2 * E + o + HPC * D)
        wqk = np.concatenate(
            [W_qkv[:, q_sl] * 0.125, W_qkv[:, k_sl]], axis=1)
        maps.append({
            "xin": xT[:, :, c * CH:(c + 1) * CH],
            "wqk": np.ascontiguousarray(
                wqk.reshape(EB, P, 2 * P).transpose(1, 0, 2)).astype(_bf16),
            "wv": np.ascontiguousarray(
                W_qkv[:, v_sl].reshape(EB, P, P).transpose(1, 0, 2)).astype(_bf16),
            "wos": W_o[c * P:(c + 1) * P, :].astype(_bf16),
            "bqk": np.stack([b_qkv[q_sl] * 0.125,
                             b_qkv[k_sl]], axis=1).astype(np.float32),
            "bv": b_qkv[v_sl].reshape(1, P).astype(_bf16),
            "bo": bo,
            "tri": tri,
        })
    return maps


def _unshard(res):
    # rank r's out[st] is global 128-token block st*8 + r
    full = np.empty((SB, P, E), _bf16)
    for r in range(NCORES):
        full[r::NCORES] = res[r]["out"]
    return full.reshape(B, S, E).astype(np.float32)


def kernel(x, W_qkv, b_qkv, W_o, b_o, mask):
    if "nc" not in _cache:
        _cache["nc"] = _build()
    nc = _cache["nc"]
    maps = _in_maps(np.asarray(x, np.float32), np.asarray(W_qkv, np.float32),
                    np.asarray(b_qkv, np.float32), np.asarray(W_o, np.float32),
                    np.asarray(b_o, np.float32))
    try:
        if "runner" not in _cache:
            _cache["runner"] = _make_runner(nc)
        res = _cache["runner"](maps)
    except Exception:
        from concourse.bass_utils import run_bass_kernel_spmd
        res = run_bass_kernel_spmd(nc, maps, list(range(NCORES))).results
    return _unshard(res)


# revision 7
# speedup vs baseline: 4.4921x; 1.2148x over previous
"""Multi-head causal self-attention (B=2, S=2048, E=1024, H=16) on 8 TRN2 NeuronCores.

Sharding: tensor-parallel over heads (2 heads/core, both batches). Per core:
  - QKV projection for its 2 heads (q^T/k^T transposed layout, v natural)
  - causal flash-style attention, scores computed transposed (k on partitions)
    so no probability-matrix transposes are needed; softmax denominator comes
    from a ones-column appended to V
  - PE-transpose of the attention output, 8-way AllToAll to reshard from
    head-columns to token-rows, then row-parallel output projection.

Host<->device traffic is minimized: x arrives token-sharded (1/8 per core,
bf16) and is AllGathered on-device; W_o arrives row-sharded and is
AllGathered on-device (its latency hides under attention); the output leaves
as bf16 and is upcast on host. The jitted PJRT callable is built once and
reused, so steady-state calls pay no jax retrace.

Emission order pipelines batch 1's QKV under batch 0's (ACT-bound) attention.
"""

import numpy as np
import ml_dtypes

P = 128
B, S, E, H, D = 2, 2048, 1024, 16, 64
NCORES = 8
EB = E // P            # 8 e-blocks
BS = B * S             # 4096 flattened rows
SBB = S // P           # 16 s-blocks per batch
SB = BS // P           # 32 s-blocks global
HPC = H // NCORES      # 2 heads per core
CH = BS // NCORES      # 512 rows owned per core

_bf16 = ml_dtypes.bfloat16
_cache = {}


def _build(no_cc=False):
    from contextlib import ExitStack

    import concourse.tile as tile
    from concourse import bacc, mybir
    from concourse.masks import make_identity

    bf16 = mybir.dt.bfloat16
    f32 = mybir.dt.float32

    nc = bacc.Bacc("TRN2", target_bir_lowering=False, debug=False,
                   num_devices=NCORES)

    grp = [list(range(NCORES))]

    # host-side layouts are [partition, eblock, col] so each load is one DMA.
    # xin: this core's 512-token column slice of x^T (tokens [512r, 512r+512)).
    xin_d = nc.dram_tensor("xin", [P, EB, CH], bf16, kind="ExternalInput")
    wqk_d = nc.dram_tensor("wqk", [P, EB, 2 * P], bf16, kind="ExternalInput")
    wv_d = nc.dram_tensor("wv", [P, EB, P], bf16, kind="ExternalInput")
    # wos: this core's 128-row slice of W_o (rows [128r, 128r+128)).
    wos_d = nc.dram_tensor("wos", [P, E], bf16, kind="ExternalInput")
    bqk_d = nc.dram_tensor("bqk", [P, 2], f32, kind="ExternalInput")
    bv_d = nc.dram_tensor("bv", [1, P], bf16, kind="ExternalInput")
    bo_d = nc.dram_tensor("bo", [1, E], bf16, kind="ExternalInput")
    tri_d = nc.dram_tensor("tri", [P, P], bf16, kind="ExternalInput")
    # rank r owns interleaved token blocks {r, r+8, r+16, r+24}: one AllToAll
    # per batch, so batch 0's A2A + output projection hide under batch 1's
    # attention. out row-block st <-> global block st*8 + rank.
    out_d = nc.dram_tensor("out", [4, P, E], bf16, kind="ExternalOutput")
    # collective bounce buffers (collectives can't touch I/O tensors).
    xag_in = nc.dram_tensor("xag_in", [P, EB, CH], bf16)
    xag_out = nc.dram_tensor("xag_out", [NCORES * P, EB, CH], bf16,
                             addr_space="Local" if no_cc else "Shared")
    wag_in = nc.dram_tensor("wag_in", [P, E], bf16)
    wag_out = nc.dram_tensor("wag_out", [NCORES * P, E], bf16,
                             addr_space="Local" if no_cc else "Shared")
    a2a_in = [nc.dram_tensor(f"a2a_in{b}", [NCORES, P, 2 * P], bf16)
              for b in range(B)]
    a2a_out = [nc.dram_tensor(f"a2a_out{b}", [NCORES, P, 2 * P], bf16)
               for b in range(B)]

    def all_gather(ins, outs):
        if no_cc:
            # cost-model variant: TimelineSim can't simulate collectives
            n = ins.shape[0]
            for j in range(NCORES):
                nc.sync.dma_start(outs[j * n:(j + 1) * n], ins[:])
        else:
            nc.gpsimd.collective_compute(
                "AllGather", mybir.AluOpType.bypass, replica_groups=grp,
                ins=[ins.ap()], outs=[outs.ap()])

    with tile.TileContext(nc) as tc, ExitStack() as ctx:
        consts = ctx.enter_context(tc.tile_pool(name="consts", bufs=1))
        work = ctx.enter_context(tc.tile_pool(name="work", bufs=1))
        xpool = ctx.enter_context(tc.tile_pool(name="xstream", bufs=2))
        epool = ctx.enter_context(tc.tile_pool(name="expst", bufs=2))
        small = ctx.enter_context(tc.tile_pool(name="small", bufs=4))
        opool = ctx.enter_context(tc.tile_pool(name="osb", bufs=2))
        pbig = ctx.enter_context(tc.tile_pool(name="pbig", bufs=2, space="PSUM"))
        ppv = ctx.enter_context(tc.tile_pool(name="ppv", bufs=2, space="PSUM"))
        psm = ctx.enter_context(tc.tile_pool(name="psm", bufs=2, space="PSUM"))

        # x AllGather first — everything downstream waits on it.
        nc.sync.dma_start(xag_in[:, :, :], xin_d[:, :, :])
        all_gather(xag_in, xag_out)
        # W_o AllGather — only needed at output projection, hides completely.
        nc.sync.dma_start(wag_in[:, :], wos_d[:, :])
        all_gather(wag_in, wag_out)

        wqk = consts.tile([P, EB, 2 * P], bf16, tag="wqk")
        wv = consts.tile([P, EB, P], bf16, tag="wv")
        wo = consts.tile([P, EB, E], bf16, tag="wo")
        bqk = consts.tile([P, 2], f32, tag="bqk")
        bv = consts.tile([1, P], bf16, tag="bv")
        bo = consts.tile([1, E], bf16, tag="bo")
        tri = consts.tile([P, P], bf16, tag="tri")
        ones1 = consts.tile([1, P], bf16, tag="ones1")
        ident = consts.tile([P, P], bf16, tag="ident")

        nc.sync.dma_start(wqk[:], wqk_d[:, :, :])
        nc.sync.dma_start(wv[:], wv_d[:, :, :])
        nc.sync.dma_start(bqk[:], bqk_d[:, :])
        nc.sync.dma_start(bv[:1, :], bv_d[:, :])
        nc.sync.dma_start(tri[:], tri_d[:, :])
        nc.vector.memset(ones1[:1, :], 1.0)
        make_identity(nc, ident[:])

        qkT = [work.tile([P, 2, S], bf16, tag=f"qkT{b}", name=f"qkT{b}")
               for b in range(B)]
        vsb = [work.tile([P, SBB, HPC, 66], bf16, tag=f"vsb{b}", name=f"vsb{b}")
               for b in range(B)]
        attn = [work.tile([P, SBB, HPC * D], bf16, tag=f"attn{b}", name=f"attn{b}")
                for b in range(B)]
        attnT = [work.tile([P, S], bf16, tag=f"attnT{b}", name=f"attnT{b}")
                 for b in range(B)]

        def qkv_pieces(b):
            """QKV projection for batch b, one 512-token chunk per piece.

            Chunks are emitted suffix-first: causal score block kb only needs
            token columns >= kb*128, so late chunks unblock the small k-blocks
            early and ACT (exp) can start before the whole projection is done.
            """
            nc.vector.memset(vsb[b][:], 1.0)
            for sc in reversed(range(S // CH)):
                g = b * (S // CH) + sc   # global 512-token chunk = rank g's shard
                xc = xpool.tile([P, EB, CH], bf16, tag="xc", name="xc")
                nc.sync.dma_start(xc[:], xag_out[g * P:(g + 1) * P, :, :])
                for db in range(2):
                    ps = psm.tile([P, CH], f32, tag="mid", name="psqk")
                    for eb in range(EB):
                        nc.tensor.matmul(
                            ps[:],
                            lhsT=wqk[:, eb, db * P:(db + 1) * P],
                            rhs=xc[:, eb, :],
                            start=(eb == 0), stop=(eb == EB - 1),
                        )
                    nc.vector.tensor_scalar_add(
                        qkT[b][:, db, sc * CH:(sc + 1) * CH], ps[:],
                        bqk[:, db:db + 1])
                    yield
                for si in range(4):
                    sb = sc * 4 + si
                    pv_ = psm.tile([P, P], f32, tag="mid", name="psv")
                    for eb in range(EB):
                        nc.tensor.matmul(
                            pv_[:], lhsT=xc[:, eb, si * P:(si + 1) * P],
                            rhs=wv[:, eb, :], start=(eb == 0), stop=False)
                    nc.tensor.matmul(pv_[:], lhsT=ones1[:1, :], rhs=bv[:1, :],
                                     start=False, stop=True)
                    nc.vector.tensor_copy(vsb[b][:, sb, 0, 0:64], pv_[:, 0:64])
                    nc.vector.tensor_copy(vsb[b][:, sb, 1, 0:64], pv_[:, 64:128])
                    yield

        def score_pieces(b, h, expst):
            """scores^T + exp for one (batch, head), one k-block per piece.

            k-blocks run high-to-low to match qkv_pieces' suffix-first order.
            """
            hs = slice(h * 64, (h + 1) * 64)
            expst.extend([None] * SBB)
            for kb in reversed(range(SBB)):
                L = S - kb * P
                # high-kb tiles are small: an extra buffer lets the next
                # batch's (reversed) scores start while this batch's PV is
                # still reading the lower k-blocks
                et = epool.tile([P, L], bf16, tag=f"e{kb}", name=f"e{kb}",
                                bufs=4 if kb >= 8 else 2)
                off = kb * P
                pos = 0
                while pos < L:  # 1024-wide psum tiles: 1 exp op per tile
                    c = min(1024, L - pos)
                    ps = pbig.tile([P, 1024], f32, tag="big", name="pssc")
                    for c0 in range(0, c, 512):
                        w = min(512, c - c0)
                        nc.tensor.matmul(
                            ps[:, c0:c0 + w],
                            lhsT=qkT[b][hs, 1, off:off + P],
                            rhs=qkT[b][hs, 0, off + pos + c0:off + pos + c0 + w],
                            start=True, stop=True)
                    nc.scalar.activation(
                        et[:, pos:pos + c], ps[:, :c],
                        mybir.ActivationFunctionType.Exp)
                    pos += c
                # zero the invalid (q < k) half of the diagonal block
                nc.gpsimd.tensor_mul(et[:, 0:P], et[:, 0:P], tri[:])
                expst[kb] = et
                yield

        def pv_pieces(b, h, expst):
            """PV + normalize for one (batch, head), one q-tile per piece."""
            hs = slice(h * 64, (h + 1) * 64)
            for qt in range(SBB):
                pp = ppv.tile([P, 66], f32, tag="pv", name="pspv")
                for kb in range(qt + 1):
                    nc.tensor.matmul(
                        pp[:, 0:65],
                        lhsT=expst[kb][:, (qt - kb) * P:(qt - kb) * P + P],
                        rhs=vsb[b][:, kb, h, 0:65],
                        start=(kb == 0), stop=(kb == qt))
                rc = small.tile([P, 1], f32, tag="recip", name="rc")
                nc.vector.reciprocal(rc[:], pp[:, 64:65])
                nc.vector.tensor_scalar_mul(
                    attn[b][:, qt, hs], pp[:, 0:64], rc[:])
                yield

        def transpose_pieces(b):
            """attn [token, e] -> attnT [e, token] via PE transposes."""
            for blk in range(SBB):
                pt = psm.tile([P, P], bf16, tag="mid", name="pst")
                nc.tensor.transpose(pt[:], attn[b][:, blk, :], ident[:])
                nc.vector.tensor_copy(attnT[b][:, blk * P:(blk + 1) * P], pt[:])
                yield
            # two strided DMAs: chunk j of the bounce gets blocks {j, j+8}
            for t in range(2):
                nc.sync.dma_start(
                    a2a_in[b].ap().rearrange(
                        "j p (t c) -> p j t c", t=2)[:, :, t, :],
                    attnT[b][:, t * NCORES * P:(t + 1) * NCORES * P].rearrange(
                        "p (j c) -> p j c", c=P))
            yield

        def interleave(*gens):
            gens = list(gens)
            while gens:
                gens = [g for g in gens if next(g, StopIteration) is not StopIteration]

        def paced(qg, score_gens, pv_gens=()):
            """Weave one qkv stream with score/pv streams, pacing emission so
            every score k-block is emitted AFTER the qkv chunk that writes the
            qkT columns it reads (Tile only tracks writer->reader deps in
            emission order). qkv chunk g (suffix-first) unlocks score k-blocks
            [12-4g, 15-4g]."""
            for g in range(4):
                for _ in range(6):
                    next(qg, None)
                for _ in range(4):
                    for sg in score_gens:
                        next(sg, None)
                    for pg in pv_gens:
                        next(pg, None)
            interleave(qg, *score_gens, *pv_gens)

        atf = [work.tile([P, EB, 2 * P], bf16, tag=f"atf{b}", name=f"atf{b}")
               for b in range(B)]

        def a2a_batch(b):
            """AllToAll batch b: head-columns -> my two token blocks."""
            if no_cc:
                # cost-model variant: TimelineSim can't simulate collectives
                for j in range(NCORES):
                    nc.sync.dma_start(a2a_out[b][j], a2a_in[b][j])
            else:
                nc.gpsimd.collective_compute(
                    "AllToAll", mybir.AluOpType.bypass,
                    replica_groups=grp,
                    ins=[a2a_in[b].ap()], outs=[a2a_out[b].ap()])
            nc.sync.dma_start(
                atf[b][:, :, :],
                a2a_out[b].ap().rearrange("j p c -> p j c"))

        def oproj_batch(b):
            """Output projection of my two token blocks of batch b."""
            for st in range(2):
                ot = opool.tile([P, E], bf16, tag="o", name="ot")
                po = pbig.tile([P, 1024], f32, tag="big", name="pso")
                for oh in range(2):
                    for eb in range(EB):
                        nc.tensor.matmul(
                            po[:, oh * 512:(oh + 1) * 512],
                            lhsT=atf[b][:, eb, st * P:(st + 1) * P],
                            rhs=wo[:, eb, oh * 512:(oh + 1) * 512],
                            start=(eb == 0), stop=False)
                    nc.tensor.matmul(po[:, oh * 512:(oh + 1) * 512],
                                     lhsT=ones1[:1, :],
                                     rhs=bo[:1, oh * 512:(oh + 1) * 512],
                                     start=False, stop=True)
                nc.vector.tensor_copy(ot[:], po[:])
                nc.sync.dma_start(out_d[b * 2 + st], ot[:])

        # ---- pipelined emission (priorities; Tile schedules by readiness) ----
        e00, e01, e10, e11 = [], [], [], []
        paced(qkv_pieces(0),
              [score_pieces(0, 0, e00), score_pieces(0, 1, e01)])
        paced(qkv_pieces(1),
              [score_pieces(1, 0, e10), score_pieces(1, 1, e11)],
              [pv_pieces(0, 0, e00), pv_pieces(0, 1, e01)])
        # full W_o materializes from the (long-finished) AllGather
        nc.sync.dma_start(wo[:], wag_out.ap().rearrange("(e p) n -> p e n", p=P))
        nc.sync.dma_start(bo[:1, :], bo_d[:, :])
        # batch-1 transposes ride along with batch-1 PV (piece qt is emitted
        # right after both heads' PV qt), so the final A2A fires early
        interleave(pv_pieces(1, 0, e10), pv_pieces(1, 1, e11),
                   transpose_pieces(0), transpose_pieces(1))
        a2a_batch(0)
        oproj_batch(0)          # hides under batch-1 attention tail
        a2a_batch(1)
        oproj_batch(1)

    nc.compile()
    return nc


def _make_runner(nc):
    """Build the shard_map'd PJRT callable ONCE; reuse across calls.

    Mirrors concourse.bass2jax.run_bass_via_pjrt, minus its per-call costs:
    the jit is cached (no retrace), inputs arrive as prebuilt global arrays
    (no per-call concat), and the donated output buffers are recycled from
    the previous call's device-side outputs (no 8.4MB of zeros on the wire;
    the kernel writes every output element so their content is irrelevant).
    """
    import jax
    import jax.numpy as jnp
    from jax.sharding import Mesh, NamedSharding, PartitionSpec
    from jax.experimental.shard_map import shard_map
    from concourse import bass2jax, mybir

    bass2jax.install_neuronx_cc_hook()
    assert nc.dbg_addr is None

    partition_name = nc.partition_id_tensor.name if nc.partition_id_tensor else None
    in_names, out_names, out_avals = [], [], []
    for alloc in nc.m.functions[0].allocations:
        if not isinstance(alloc, mybir.MemoryLocationSet):
            continue
        name = alloc.memorylocations[0].name
        if alloc.kind == "ExternalInput":
            if name != partition_name:
                in_names.append(name)
        elif alloc.kind == "ExternalOutput":
            out_names.append(name)
            shape = tuple(alloc.tensor_shape)
            dtype = mybir.dt.np(alloc.dtype)
            out_avals.append(jax.core.ShapedArray(shape, dtype))
    n_params = len(in_names)
    n_outs = len(out_avals)
    all_names = in_names + out_names + ([partition_name] if partition_name else [])
    donate = tuple(range(n_params, n_params + n_outs))

    def _body(*args):
        operands = list(args)
        if partition_name is not None:
            operands.append(bass2jax.partition_id_tensor())
        outs = bass2jax._bass_exec_p.bind(
            *operands,
            out_avals=tuple(out_avals),
            in_names=tuple(all_names),
            out_names=tuple(out_names),
            lowering_input_output_aliases=(),
            sim_require_finite=True,
            sim_require_nnan=True,
            nc=nc,
        )
        return tuple(outs)

    devices = jax.devices()[:NCORES]
    mesh = Mesh(np.asarray(devices), ("core",))
    spec = NamedSharding(mesh, PartitionSpec("core"))
    in_specs = (PartitionSpec("core"),) * (n_params + n_outs)
    out_specs = (PartitionSpec("core"),) * n_outs
    sharded = jax.jit(
        shard_map(_body, mesh=mesh, in_specs=in_specs, out_specs=out_specs,
                  check_rep=False),
        donate_argnums=donate, keep_unused=True)
    gshapes = [(NCORES * a.shape[0], *a.shape[1:]) for a in out_avals]
    mkzeros = jax.jit(
        lambda: tuple(jnp.zeros(s, a.dtype) for s, a in zip(gshapes, out_avals)),
        out_shardings=(spec,) * n_outs)
    state = {"don": None}

    def run(maps):
        gin = [maps[name] for name in in_names]
        don = state["don"]
        state["don"] = None
        if don is None:
            don = mkzeros()
        out_arrs = sharded(*gin, *don)
        res = [
            {name: np.asarray(out_arrs[i]).reshape(NCORES, *out_avals[i].shape)[c]
             for i, name in enumerate(out_names)}
            for c in range(NCORES)
        ]
        state["don"] = list(out_arrs)
        return res

    return run


def _in_maps(x, W_qkv, b_qkv, W_o, b_o):
    """Global [8*d0, ...] arrays (core c's shard = rows [c*d0, (c+1)*d0))."""
    # xin[c*128+p, e, t] = x[c*512+t, e*128+p]: core c's x^T token slice
    xin = np.ascontiguousarray(
        x.reshape(NCORES, CH, EB, P).transpose(0, 3, 2, 1)).astype(_bf16)
    wqk = np.empty((NCORES * P, EB, 2 * P), _bf16)
    wv = np.empty((NCORES * P, EB, P), _bf16)
    bqk = np.empty((NCORES * P, 2), np.float32)
    for c in range(NCORES):
        o = c * HPC * D
        q_sl = slice(o, o + HPC * D)
        k_sl = slice(E + o, E + o + HPC * D)
        v_sl = slice(2 * E + o, 2 * E + o + HPC * D)
        wqk_c = np.concatenate(
            [W_qkv[:, q_sl] * 0.125, W_qkv[:, k_sl]], axis=1)
        wqk[c * P:(c + 1) * P] = wqk_c.reshape(EB, P, 2 * P).transpose(1, 0, 2)
        wv[c * P:(c + 1) * P] = (
            W_qkv[:, v_sl].reshape(EB, P, P).transpose(1, 0, 2))
        bqk[c * P:(c + 1) * P] = np.stack(
            [b_qkv[q_sl] * 0.125, b_qkv[k_sl]], axis=1)
    return {
        "xin": xin.reshape(NCORES * P, EB, CH),
        "wqk": wqk,
        "wv": wv,
        "wos": W_o.astype(_bf16),
        "bqk": bqk,
        "bv": b_qkv[2 * E:].reshape(NCORES, P).astype(_bf16),
        "bo": np.tile(b_o.reshape(1, E), (NCORES, 1)).astype(_bf16),
        "tri": np.tile(np.triu(np.ones((P, P), np.float32)), (NCORES, 1)).astype(_bf16),
    }


def _unshard(res):
    # rank r's out[st] is global 128-token block st*8 + r
    full = np.empty((SB, P, E), _bf16)
    for r in range(NCORES):
        full[r::NCORES] = res[r]["out"]
    return full.reshape(B, S, E).astype(np.float32)


def kernel(x, W_qkv, b_qkv, W_o, b_o, mask):
    if "nc" not in _cache:
        _cache["nc"] = _build()
    nc = _cache["nc"]
    maps = _in_maps(np.asarray(x, np.float32), np.asarray(W_qkv, np.float32),
                    np.asarray(b_qkv, np.float32), np.asarray(W_o, np.float32),
                    np.asarray(b_o, np.float32))
    try:
        if "runner" not in _cache:
            _cache["runner"] = _make_runner(nc)
        res = _cache["runner"](maps)
    except Exception:
        from concourse.bass_utils import run_bass_kernel_spmd
        per_core = [
            {k: v[c * (v.shape[0] // NCORES):(c + 1) * (v.shape[0] // NCORES)]
             for k, v in maps.items()}
            for c in range(NCORES)
        ]
        res = run_bass_kernel_spmd(nc, per_core, list(range(NCORES))).results
    return _unshard(res)


# revision 10
# speedup vs baseline: 4.5440x; 1.0116x over previous
"""Multi-head causal self-attention (B=2, S=2048, E=1024, H=16) on 8 TRN2 NeuronCores.

Sharding: tensor-parallel over heads (2 heads/core, both batches). Per core:
  - QKV projection for its 2 heads (q^T/k^T transposed layout, v natural)
  - causal flash-style attention, scores computed transposed (k on partitions)
    so no probability-matrix transposes are needed; softmax denominator comes
    from a ones-column appended to V
  - PE-transpose of the attention output, 8-way AllToAll to reshard from
    head-columns to token-rows, then row-parallel output projection.

Host<->device traffic is minimized: x arrives token-sharded (1/8 per core,
bf16) and is AllGathered on-device; W_o arrives row-sharded and is
AllGathered on-device (its latency hides under attention); the output leaves
as bf16 and is upcast on host. The jitted PJRT callable is built once and
reused, so steady-state calls pay no jax retrace.

Emission order pipelines batch 1's QKV under batch 0's (ACT-bound) attention.
"""

import numpy as np
import ml_dtypes

P = 128
B, S, E, H, D = 2, 2048, 1024, 16, 64
NCORES = 8
EB = E // P            # 8 e-blocks
BS = B * S             # 4096 flattened rows
SBB = S // P           # 16 s-blocks per batch
SB = BS // P           # 32 s-blocks global
HPC = H // NCORES      # 2 heads per core
CH = BS // NCORES      # 512 rows owned per core

_bf16 = ml_dtypes.bfloat16
_cache = {}


def _build(no_cc=False):
    from contextlib import ExitStack

    import concourse.tile as tile
    from concourse import bacc, mybir
    from concourse.masks import make_identity

    bf16 = mybir.dt.bfloat16
    f32 = mybir.dt.float32

    nc = bacc.Bacc("TRN2", target_bir_lowering=False, debug=False,
                   num_devices=NCORES)

    grp = [list(range(NCORES))]

    # host-side layouts are [partition, eblock, col] so each load is one DMA.
    # xin: this core's 512-token column slice of x^T (tokens [512r, 512r+512)).
    xin_d = nc.dram_tensor("xin", [P, EB, CH], bf16, kind="ExternalInput")
    wqk_d = nc.dram_tensor("wqk", [P, EB, 2 * P], bf16, kind="ExternalInput")
    wv_d = nc.dram_tensor("wv", [P, EB, P], bf16, kind="ExternalInput")
    # wos: this core's 128-row slice of W_o (rows [128r, 128r+128)).
    wos_d = nc.dram_tensor("wos", [P, E], bf16, kind="ExternalInput")
    # packed small inputs: aux = bqk (2 cols) | tri (128 cols), per partition;
    # auxr row 0 = bv (128) | bo (1024)
    aux_d = nc.dram_tensor("aux", [P, 2 + P], bf16, kind="ExternalInput")
    auxr_d = nc.dram_tensor("auxr", [1, P + E], bf16, kind="ExternalInput")
    # rank r owns interleaved token blocks {r, r+8, r+16, r+24}: one AllToAll
    # per batch, so batch 0's A2A + output projection hide under batch 1's
    # attention. out row-block st <-> global block st*8 + rank.
    out_d = nc.dram_tensor("out", [4, P, E], bf16, kind="ExternalOutput")
    # collective bounce buffers (collectives can't touch I/O tensors).
    xag_in = nc.dram_tensor("xag_in", [P, EB, CH], bf16)
    xag_out = nc.dram_tensor("xag_out", [NCORES * P, EB, CH], bf16,
                             addr_space="Local" if no_cc else "Shared")
    wag_in = nc.dram_tensor("wag_in", [P, E], bf16)
    wag_out = nc.dram_tensor("wag_out", [NCORES * P, E], bf16,
                             addr_space="Local" if no_cc else "Shared")
    a2a_in = [nc.dram_tensor(f"a2a_in{b}", [NCORES, P, 2 * P], bf16)
              for b in range(B)]
    a2a_out = [nc.dram_tensor(f"a2a_out{b}", [NCORES, P, 2 * P], bf16)
               for b in range(B)]

    def all_gather(ins, outs):
        if no_cc:
            # cost-model variant: TimelineSim can't simulate collectives
            n = ins.shape[0]
            for j in range(NCORES):
                nc.sync.dma_start(outs[j * n:(j + 1) * n], ins[:])
        else:
            nc.gpsimd.collective_compute(
                "AllGather", mybir.AluOpType.bypass, replica_groups=grp,
                ins=[ins.ap()], outs=[outs.ap()])

    with tile.TileContext(nc) as tc, ExitStack() as ctx:
        consts = ctx.enter_context(tc.tile_pool(name="consts", bufs=1))
        work = ctx.enter_context(tc.tile_pool(name="work", bufs=1))
        xpool = ctx.enter_context(tc.tile_pool(name="xstream", bufs=2))
        epool = ctx.enter_context(tc.tile_pool(name="expst", bufs=2))
        small = ctx.enter_context(tc.tile_pool(name="small", bufs=4))
        opool = ctx.enter_context(tc.tile_pool(name="osb", bufs=2))
        pbig = ctx.enter_context(tc.tile_pool(name="pbig", bufs=2, space="PSUM"))
        ppv = ctx.enter_context(tc.tile_pool(name="ppv", bufs=2, space="PSUM"))
        psm = ctx.enter_context(tc.tile_pool(name="psm", bufs=2, space="PSUM"))

        # x AllGather first — everything downstream waits on it.
        nc.sync.dma_start(xag_in[:, :, :], xin_d[:, :, :])
        all_gather(xag_in, xag_out)
        # W_o AllGather — only needed at output projection, hides completely.
        nc.sync.dma_start(wag_in[:, :], wos_d[:, :])
        all_gather(wag_in, wag_out)

        wqk = consts.tile([P, EB, 2 * P], bf16, tag="wqk")
        wv = consts.tile([P, EB, P], bf16, tag="wv")
        wo = consts.tile([P, EB, E], bf16, tag="wo")
        bqk = consts.tile([P, 2], f32, tag="bqk")
        bv = consts.tile([1, P], bf16, tag="bv")
        bo = consts.tile([1, E], bf16, tag="bo")
        tri = consts.tile([P, P], bf16, tag="tri")
        ones1 = consts.tile([1, P], bf16, tag="ones1")
        ident = consts.tile([P, P], bf16, tag="ident")

        nc.sync.dma_start(wqk[:], wqk_d[:, :, :])
        nc.sync.dma_start(wv[:], wv_d[:, :, :])
        nc.sync.dma_start(bqk[:], bqk_d[:, :])
        nc.sync.dma_start(bv[:1, :], bv_d[:, :])
        nc.sync.dma_start(tri[:], tri_d[:, :])
        nc.vector.memset(ones1[:1, :], 1.0)
        make_identity(nc, ident[:])

        qkT = [work.tile([P, 2, S], bf16, tag=f"qkT{b}", name=f"qkT{b}")
               for b in range(B)]
        vsb = [work.tile([P, SBB, HPC, 66], bf16, tag=f"vsb{b}", name=f"vsb{b}")
               for b in range(B)]
        attn = [work.tile([P, SBB, HPC * D], bf16, tag=f"attn{b}", name=f"attn{b}")
                for b in range(B)]
        attnT = [work.tile([P, S], bf16, tag=f"attnT{b}", name=f"attnT{b}")
                 for b in range(B)]

        def qkv_pieces(b):
            """QKV projection for batch b, one 512-token chunk per piece.

            Chunks are emitted suffix-first: causal score block kb only needs
            token columns >= kb*128, so late chunks unblock the small k-blocks
            early and ACT (exp) can start before the whole projection is done.
            """
            nc.vector.memset(vsb[b][:], 1.0)
            for sc in reversed(range(S // CH)):
                g = b * (S // CH) + sc   # global 512-token chunk = rank g's shard
                xc = xpool.tile([P, EB, CH], bf16, tag="xc", name="xc")
                nc.sync.dma_start(xc[:], xag_out[g * P:(g + 1) * P, :, :])
                for db in range(2):
                    ps = psm.tile([P, CH], f32, tag="mid", name="psqk")
                    for eb in range(EB):
                        nc.tensor.matmul(
                            ps[:],
                            lhsT=wqk[:, eb, db * P:(db + 1) * P],
                            rhs=xc[:, eb, :],
                            start=(eb == 0), stop=(eb == EB - 1),
                        )
                    nc.vector.tensor_scalar_add(
                        qkT[b][:, db, sc * CH:(sc + 1) * CH], ps[:],
                        bqk[:, db:db + 1])
                    yield
                for si in range(4):
                    sb = sc * 4 + si
                    pv_ = psm.tile([P, P], f32, tag="mid", name="psv")
                    for eb in range(EB):
                        nc.tensor.matmul(
                            pv_[:], lhsT=xc[:, eb, si * P:(si + 1) * P],
                            rhs=wv[:, eb, :], start=(eb == 0), stop=False)
                    nc.tensor.matmul(pv_[:], lhsT=ones1[:1, :], rhs=bv[:1, :],
                                     start=False, stop=True)
                    nc.vector.tensor_copy(vsb[b][:, sb, 0, 0:64], pv_[:, 0:64])
                    nc.vector.tensor_copy(vsb[b][:, sb, 1, 0:64], pv_[:, 64:128])
                    yield

        def score_pieces(b, expst0, expst1):
            """scores^T + exp for BOTH heads of one batch, one k-block per
            piece. The heads' K=64 matmuls are emitted adjacently: head0 sits
            on partitions 0-63 and head1 on 64-127, so their inferred PE
            tile_positions are (0,0)/(64,0) and the array runs each pair
            nearly concurrently in its two row halves.

            k-blocks run high-to-low to match qkv_pieces' suffix-first order.
            """
            expst0.extend([None] * SBB)
            expst1.extend([None] * SBB)
            for kb in reversed(range(SBB)):
                L = S - kb * P
                # high-kb tiles are small: an extra buffer lets the next
                # batch's (reversed) scores start while this batch's PV is
                # still reading the lower k-blocks
                et = [epool.tile([P, L], bf16, tag=f"e{kb}", name=f"e{kb}",
                                 bufs=4 if kb >= 8 else 2)
                      for _ in range(2)]
                off = kb * P
                pos = 0
                while pos < L:  # 1024-wide psum tiles: 1 exp op per tile
                    c = min(1024, L - pos)
                    ps = [pbig.tile([P, 1024], f32, tag="big", name="pssc")
                          for _ in range(2)]
                    for c0 in range(0, c, 512):
                        w = min(512, c - c0)
                        for h in range(2):
                            hs = slice(h * 64, (h + 1) * 64)
                            nc.tensor.matmul(
                                ps[h][:, c0:c0 + w],
                                lhsT=qkT[b][hs, 1, off:off + P],
                                rhs=qkT[b][hs, 0,
                                           off + pos + c0:off + pos + c0 + w],
                                start=True, stop=True)
                    for h in range(2):
                        nc.scalar.activation(
                            et[h][:, pos:pos + c], ps[h][:, :c],
                            mybir.ActivationFunctionType.Exp)
                    pos += c
                # zero the invalid (q < k) half of the diagonal block
                for h in range(2):
                    nc.gpsimd.tensor_mul(et[h][:, 0:P], et[h][:, 0:P], tri[:])
                expst0[kb] = et[0]
                expst1[kb] = et[1]
                yield

        def pv_pieces(b, h, expst):
            """PV + normalize for one (batch, head), one q-tile per piece."""
            hs = slice(h * 64, (h + 1) * 64)
            for qt in range(SBB):
                pp = ppv.tile([P, 66], f32, tag="pv", name="pspv")
                for kb in range(qt + 1):
                    nc.tensor.matmul(
                        pp[:, 0:65],
                        lhsT=expst[kb][:, (qt - kb) * P:(qt - kb) * P + P],
                        rhs=vsb[b][:, kb, h, 0:65],
                        start=(kb == 0), stop=(kb == qt))
                rc = small.tile([P, 1], f32, tag="recip", name="rc")
                nc.vector.reciprocal(rc[:], pp[:, 64:65])
                nc.vector.tensor_scalar_mul(
                    attn[b][:, qt, hs], pp[:, 0:64], rc[:])
                yield

        def transpose_pieces(b):
            """attn [token, e] -> attnT [e, token] via PE transposes."""
            for blk in range(SBB):
                pt = psm.tile([P, P], bf16, tag="mid", name="pst")
                nc.tensor.transpose(pt[:], attn[b][:, blk, :], ident[:])
                nc.vector.tensor_copy(attnT[b][:, blk * P:(blk + 1) * P], pt[:])
                yield
            # two strided DMAs: chunk j of the bounce gets blocks {j, j+8}
            for t in range(2):
                nc.sync.dma_start(
                    a2a_in[b].ap().rearrange(
                        "j p (t c) -> p j t c", t=2)[:, :, t, :],
                    attnT[b][:, t * NCORES * P:(t + 1) * NCORES * P].rearrange(
                        "p (j c) -> p j c", c=P))
            yield

        def interleave(*gens):
            gens = list(gens)
            while gens:
                gens = [g for g in gens if next(g, StopIteration) is not StopIteration]

        def paced(qg, score_gens, pv_gens=()):
            """Weave one qkv stream with score/pv streams, pacing emission so
            every score k-block is emitted AFTER the qkv chunk that writes the
            qkT columns it reads (Tile only tracks writer->reader deps in
            emission order). qkv chunk g (suffix-first) unlocks score k-blocks
            [12-4g, 15-4g]."""
            for g in range(4):
                for _ in range(6):
                    next(qg, None)
                for _ in range(4):
                    for sg in score_gens:
                        next(sg, None)
                    for pg in pv_gens:
                        next(pg, None)
            interleave(qg, *score_gens, *pv_gens)

        atf = [work.tile([P, EB, 2 * P], bf16, tag=f"atf{b}", name=f"atf{b}")
               for b in range(B)]

        def a2a_batch(b):
            """AllToAll batch b: head-columns -> my two token blocks."""
            if no_cc:
                # cost-model variant: TimelineSim can't simulate collectives
                for j in range(NCORES):
                    nc.sync.dma_start(a2a_out[b][j], a2a_in[b][j])
            else:
                nc.gpsimd.collective_compute(
                    "AllToAll", mybir.AluOpType.bypass,
                    replica_groups=grp,
                    ins=[a2a_in[b].ap()], outs=[a2a_out[b].ap()])
            nc.sync.dma_start(
                atf[b][:, :, :],
                a2a_out[b].ap().rearrange("j p c -> p j c"))

        def oproj_batch(b):
            """Output projection of my two token blocks of batch b."""
            for st in range(2):
                ot = opool.tile([P, E], bf16, tag="o", name="ot")
                po = pbig.tile([P, 1024], f32, tag="big", name="pso")
                for oh in range(2):
                    for eb in range(EB):
                        nc.tensor.matmul(
                            po[:, oh * 512:(oh + 1) * 512],
                            lhsT=atf[b][:, eb, st * P:(st + 1) * P],
                            rhs=wo[:, eb, oh * 512:(oh + 1) * 512],
                            start=(eb == 0), stop=False)
                    nc.tensor.matmul(po[:, oh * 512:(oh + 1) * 512],
                                     lhsT=ones1[:1, :],
                                     rhs=bo[:1, oh * 512:(oh + 1) * 512],
                                     start=False, stop=True)
                nc.vector.tensor_copy(ot[:], po[:])
                nc.sync.dma_start(out_d[b * 2 + st], ot[:])

        # ---- pipelined emission (priorities; Tile schedules by readiness) ----
        e00, e01, e10, e11 = [], [], [], []
        paced(qkv_pieces(0), [score_pieces(0, e00, e01)])
        paced(qkv_pieces(1),
              [score_pieces(1, e10, e11)],
              [pv_pieces(0, 0, e00), pv_pieces(0, 1, e01)])
        # full W_o materializes from the (long-finished) AllGather
        nc.sync.dma_start(wo[:], wag_out.ap().rearrange("(e p) n -> p e n", p=P))
        nc.sync.dma_start(bo[:1, :], bo_d[:, :])
        # batch-1 transposes ride along with batch-1 PV (piece qt is emitted
        # right after both heads' PV qt), so the final A2A fires early
        interleave(pv_pieces(1, 0, e10), pv_pieces(1, 1, e11),
                   transpose_pieces(0), transpose_pieces(1))
        a2a_batch(0)
        oproj_batch(0)          # hides under batch-1 attention tail
        a2a_batch(1)
        oproj_batch(1)

    nc.compile()
    return nc


def _make_runner(nc):
    """Build the shard_map'd PJRT callable ONCE; reuse across calls.

    Mirrors concourse.bass2jax.run_bass_via_pjrt, minus its per-call costs:
    the jit is cached (no retrace), inputs arrive as prebuilt global arrays
    (no per-call concat), and the donated output buffers are recycled from
    the previous call's device-side outputs (no 8.4MB of zeros on the wire;
    the kernel writes every output element so their content is irrelevant).
    """
    import jax
    import jax.numpy as jnp
    from jax.sharding import Mesh, NamedSharding, PartitionSpec
    from jax.experimental.shard_map import shard_map
    from concourse import bass2jax, mybir

    bass2jax.install_neuronx_cc_hook()
    assert nc.dbg_addr is None

    partition_name = nc.partition_id_tensor.name if nc.partition_id_tensor else None
    in_names, out_names, out_avals = [], [], []
    for alloc in nc.m.functions[0].allocations:
        if not isinstance(alloc, mybir.MemoryLocationSet):
            continue
        name = alloc.memorylocations[0].name
        if alloc.kind == "ExternalInput":
            if name != partition_name:
                in_names.append(name)
        elif alloc.kind == "ExternalOutput":
            out_names.append(name)
            shape = tuple(alloc.tensor_shape)
            dtype = mybir.dt.np(alloc.dtype)
            out_avals.append(jax.core.ShapedArray(shape, dtype))
    n_params = len(in_names)
    n_outs = len(out_avals)
    all_names = in_names + out_names + ([partition_name] if partition_name else [])
    donate = tuple(range(n_params, n_params + n_outs))

    def _body(*args):
        operands = list(args)
        if partition_name is not None:
            operands.append(bass2jax.partition_id_tensor())
        outs = bass2jax._bass_exec_p.bind(
            *operands,
            out_avals=tuple(out_avals),
            in_names=tuple(all_names),
            out_names=tuple(out_names),
            lowering_input_output_aliases=(),
            sim_require_finite=True,
            sim_require_nnan=True,
            nc=nc,
        )
        return tuple(outs)

    devices = jax.devices()[:NCORES]
    mesh = Mesh(np.asarray(devices), ("core",))
    spec = NamedSharding(mesh, PartitionSpec("core"))
    in_specs = (PartitionSpec("core"),) * (n_params + n_outs)
    out_specs = (PartitionSpec("core"),) * n_outs
    sharded = jax.jit(
        shard_map(_body, mesh=mesh, in_specs=in_specs, out_specs=out_specs,
                  check_rep=False),
        donate_argnums=donate, keep_unused=True)
    gshapes = [(NCORES * a.shape[0], *a.shape[1:]) for a in out_avals]
    mkzeros = jax.jit(
        lambda: tuple(jnp.zeros(s, a.dtype) for s, a in zip(gshapes, out_avals)),
        out_shardings=(spec,) * n_outs)
    state = {"don": None}

    def run(maps):
        gin = [maps[name] for name in in_names]
        don = state["don"]
        state["don"] = None
        if don is None:
            don = mkzeros()
        out_arrs = sharded(*gin, *don)
        res = [
            {name: np.asarray(out_arrs[i]).reshape(NCORES, *out_avals[i].shape)[c]
             for i, name in enumerate(out_names)}
            for c in range(NCORES)
        ]
        state["don"] = list(out_arrs)
        return res

    return run


def _in_maps(x, W_qkv, b_qkv, W_o, b_o):
    """Global [8*d0, ...] arrays (core c's shard = rows [c*d0, (c+1)*d0))."""
    # xin[c*128+p, e, t] = x[c*512+t, e*128+p]: core c's x^T token slice
    xin = np.ascontiguousarray(
        x.reshape(NCORES, CH, EB, P).transpose(0, 3, 2, 1)).astype(_bf16)
    wqk = np.empty((NCORES * P, EB, 2 * P), _bf16)
    wv = np.empty((NCORES * P, EB, P), _bf16)
    bqk = np.empty((NCORES * P, 2), np.float32)
    for c in range(NCORES):
        o = c * HPC * D
        q_sl = slice(o, o + HPC * D)
        k_sl = slice(E + o, E + o + HPC * D)
        v_sl = slice(2 * E + o, 2 * E + o + HPC * D)
        wqk_c = np.concatenate(
            [W_qkv[:, q_sl] * 0.125, W_qkv[:, k_sl]], axis=1)
        wqk[c * P:(c + 1) * P] = wqk_c.reshape(EB, P, 2 * P).transpose(1, 0, 2)
        wv[c * P:(c + 1) * P] = (
            W_qkv[:, v_sl].reshape(EB, P, P).transpose(1, 0, 2))
        bqk[c * P:(c + 1) * P] = np.stack(
            [b_qkv[q_sl] * 0.125, b_qkv[k_sl]], axis=1)
    return {
        "xin": xin.reshape(NCORES * P, EB, CH),
        "wqk": wqk,
        "wv": wv,
        "wos": W_o.astype(_bf16),
        "bqk": bqk,
        "bv": b_qkv[2 * E:].reshape(NCORES, P).astype(_bf16),
        "bo": np.tile(b_o.reshape(1, E), (NCORES, 1)).astype(_bf16),
        "tri": np.tile(np.triu(np.ones((P, P), np.float32)), (NCORES, 1)).astype(_bf16),
    }


def _unshard(res):
    # rank r's out[st] is global 128-token block st*8 + r
    full = np.empty((SB, P, E), _bf16)
    for r in range(NCORES):
        full[r::NCORES] = res[r]["out"]
    return full.reshape(B, S, E).astype(np.float32)


def kernel(x, W_qkv, b_qkv, W_o, b_o, mask):
    if "nc" not in _cache:
        _cache["nc"] = _build()
    nc = _cache["nc"]
    maps = _in_maps(np.asarray(x, np.float32), np.asarray(W_qkv, np.float32),
                    np.asarray(b_qkv, np.float32), np.asarray(W_o, np.float32),
                    np.asarray(b_o, np.float32))
    try:
        if "runner" not in _cache:
            _cache["runner"] = _make_runner(nc)
        res = _cache["runner"](maps)
    except Exception:
        from concourse.bass_utils import run_bass_kernel_spmd
        per_core = [
            {k: v[c * (v.shape[0] // NCORES):(c + 1) * (v.shape[0] // NCORES)]
             for k, v in maps.items()}
            for c in range(NCORES)
        ]
        res = run_bass_kernel_spmd(nc, per_core, list(range(NCORES))).results
    return _unshard(res)


# revision 30
# speedup vs baseline: 4.6685x; 1.0274x over previous
"""Multi-head causal self-attention (B=2, S=2048, E=1024, H=16) on 8 TRN2 NeuronCores.

Sharding: tensor-parallel over heads (2 heads/core, both batches). Per core:
  - QKV projection for its 2 heads (q^T/k^T transposed layout, v natural)
  - causal flash-style attention, scores computed transposed (k on partitions)
    so no probability-matrix transposes are needed; softmax denominator comes
    from a ones-column appended to V
  - PE-transpose of the attention output, 8-way AllToAll to reshard from
    head-columns to token-rows, then row-parallel output projection.

Host<->device traffic is minimized: x arrives token-sharded (1/8 per core,
bf16) and is AllGathered on-device; W_o arrives row-sharded and is
AllGathered on-device (its latency hides under attention); the output leaves
as bf16 and is upcast on host. The jitted PJRT callable is built once and
reused, so steady-state calls pay no jax retrace.

Emission order pipelines batch 1's QKV under batch 0's (ACT-bound) attention.
"""

import numpy as np
import ml_dtypes

P = 128
B, S, E, H, D = 2, 2048, 1024, 16, 64
NCORES = 8
EB = E // P            # 8 e-blocks
BS = B * S             # 4096 flattened rows
SBB = S // P           # 16 s-blocks per batch
SB = BS // P           # 32 s-blocks global
HPC = H // NCORES      # 2 heads per core
CH = BS // NCORES      # 512 rows owned per core
NAG = 4                # x AllGather column chunks

_bf16 = ml_dtypes.bfloat16
_cache = {}


def _build(no_cc=False):
    from contextlib import ExitStack

    import concourse.tile as tile
    from concourse import bacc, mybir
    from concourse.masks import make_identity

    bf16 = mybir.dt.bfloat16
    f32 = mybir.dt.float32

    nc = bacc.Bacc("TRN2", target_bir_lowering=False, debug=False,
                   num_devices=NCORES)

    grp = [list(range(NCORES))]

    # host-side layouts are [partition, eblock, col] so each load is one DMA.
    # xin: this core's x^T column slice — col k*128+c = token block rank+8k
    # (interleaved so AG chunk k covers the contiguous tokens [1024k, ...)).
    xin_d = nc.dram_tensor("xin", [P, EB, CH], bf16, kind="ExternalInput")
    # winp packs this core's weights per partition row: wqk [EB,2P] flat
    # (cols 0:2048) | wv [EB,P] flat (2048:3072) | its W_o row-slice
    # W_o[128r:128r+128, :] (3072:4096)
    winp_d = nc.dram_tensor("winp", [P, 4 * E], bf16, kind="ExternalInput")
    # packed small inputs: aux = bqk (2 cols) | tri (128 cols), per partition;
    # auxr row 0 = bv (128) | bo (1024)
    aux_d = nc.dram_tensor("aux", [P, 2 + P], bf16, kind="ExternalInput")
    auxr_d = nc.dram_tensor("auxr", [1, P + E], bf16, kind="ExternalInput")
    # rank r owns interleaved token blocks {r, r+8, r+16, r+24}: one AllToAll
    # per batch, so batch 0's A2A + output projection hide under batch 1's
    # attention. out row-block st <-> global block st*8 + rank.
    out_d = nc.dram_tensor("out", [4, P, E], bf16, kind="ExternalOutput")
    # collective bounce buffers (collectives can't touch I/O tensors).
    # x is gathered in 4 column chunks; with the interleaved host sharding
    # (xin col k*128+c = token block rank+8k) chunk i delivers the contiguous
    # global token span [1024*i, 1024*(i+1)) so the gather pipelines under
    # the suffix-first QKV/attention schedule instead of blocking it.
    xag_in = [nc.dram_tensor(f"xag_in{i}", [P, EB, P], bf16)
              for i in range(NAG)]
    xag_out = [nc.dram_tensor(f"xag_out{i}", [NCORES * P, EB, P], bf16,
                              addr_space="Local" if no_cc else "Shared")
               for i in range(NAG)]
    wag_in = nc.dram_tensor("wag_in", [P, E], bf16)
    wag_out = nc.dram_tensor("wag_out", [NCORES * P, E], bf16,
                             addr_space="Local" if no_cc else "Shared")
    a2a_in = [nc.dram_tensor(f"a2a_in{b}", [NCORES, P, 2 * P], bf16)
              for b in range(B)]
    a2a_out = [nc.dram_tensor(f"a2a_out{b}", [NCORES, P, 2 * P], bf16)
               for b in range(B)]

    def all_gather(ins, outs):
        if no_cc:
            # cost-model variant: TimelineSim can't simulate collectives
            n = ins.shape[0]
            for j in range(NCORES):
                nc.sync.dma_start(outs[j * n:(j + 1) * n], ins[:])
        else:
            nc.gpsimd.collective_compute(
                "AllGather", mybir.AluOpType.bypass, replica_groups=grp,
                ins=[ins.ap()], outs=[outs.ap()])

    with tile.TileContext(nc) as tc, ExitStack() as ctx:
        consts = ctx.enter_context(tc.tile_pool(name="consts", bufs=1))
        work = ctx.enter_context(tc.tile_pool(name="work", bufs=1))
        xpool = ctx.enter_context(tc.tile_pool(name="xstream", bufs=2))
        epool = ctx.enter_context(tc.tile_pool(name="expst", bufs=2))
        small = ctx.enter_context(tc.tile_pool(name="small", bufs=4))
        opool = ctx.enter_context(tc.tile_pool(name="osb", bufs=2))
        pbig = ctx.enter_context(tc.tile_pool(name="pbig", bufs=2, space="PSUM"))
        ppv = ctx.enter_context(tc.tile_pool(name="ppv", bufs=2, space="PSUM"))
        psm = ctx.enter_context(tc.tile_pool(name="psm", bufs=2, space="PSUM"))

        # x AllGathers first, in the order the suffix-first schedule consumes
        # them: batch 0's late tokens (chunk 1), early (0), then batch 1's
        # (3, 2). W_o's gather rides last; it's only needed at output
        # projection.
        for i in (1, 0, 3, 2):
            nc.sync.dma_start(xag_in[i][:, :, :],
                              xin_d[:, :, i * P:(i + 1) * P])
            all_gather(xag_in[i], xag_out[i])
        nc.sync.dma_start(wag_in[:, :], winp_d[:, 3 * E:4 * E])
        all_gather(wag_in, wag_out)

        wqk = consts.tile([P, EB, 2 * P], bf16, tag="wqk")
        wv = consts.tile([P, EB, P], bf16, tag="wv")
        wo = consts.tile([P, EB, E], bf16, tag="wo")
        aux = consts.tile([P, 2 + P], bf16, tag="aux")
        auxr = consts.tile([1, P + E], bf16, tag="auxr")
        ones1 = consts.tile([1, P], bf16, tag="ones1")
        ident = consts.tile([P, P], bf16, tag="ident")

        nc.sync.dma_start(
            wqk[:], winp_d.ap()[:, 0:2 * E].rearrange("p (e c) -> p e c", c=2 * P))
        nc.sync.dma_start(
            wv[:], winp_d.ap()[:, 2 * E:3 * E].rearrange("p (e c) -> p e c", c=P))
        nc.sync.dma_start(aux[:], aux_d[:, :])
        nc.sync.dma_start(auxr[:1, :], auxr_d[:, :])
        nc.vector.memset(ones1[:1, :], 1.0)
        make_identity(nc, ident[:])
        bqkf = consts.tile([P, 2], f32, tag="bqkf")
        nc.vector.tensor_copy(bqkf[:], aux[:, 0:2])  # scalar op needs f32

        qkT = [work.tile([P, 2, S], bf16, tag=f"qkT{b}", name=f"qkT{b}")
               for b in range(B)]
        vsb = [work.tile([P, SBB, HPC, 66], bf16, tag=f"vsb{b}", name=f"vsb{b}")
               for b in range(B)]
        attn = [work.tile([P, SBB, HPC * D], bf16, tag=f"attn{b}", name=f"attn{b}")
                for b in range(B)]
        attnT = [work.tile([P, S], bf16, tag=f"attnT{b}", name=f"attnT{b}")
                 for b in range(B)]

        def qkv_pieces(b):
            """QKV projection for batch b, one 512-token chunk per piece.

            Chunks are emitted suffix-first: causal score block kb only needs
            token columns >= kb*128, so late chunks unblock the small k-blocks
            early and ACT (exp) can start before the whole projection is done.
            """
            nc.vector.memset(vsb[b][:], 1.0)
            for sc in reversed(range(S // CH)):
                i = 2 * b + sc // 2     # AG chunk holding this 512-token chunk
                xc = xpool.tile([P, 4, EB, P], bf16, tag="xc", name="xc")
                for t in range(4):
                    j = 4 * (sc % 2) + t    # rank row inside the AG chunk
                    nc.sync.dma_start(xc[:, t, :, :],
                                      xag_out[i][j * P:(j + 1) * P, :, :])
                for db in range(2):
                    ps = psm.tile([P, CH], f32, tag="mid", name="psqk")
                    # t-outer: one open PSUM accumulation group per zero
                    # region at a time (interleaving groups is illegal)
                    for t in range(4):
                        for eb in range(EB):
                            nc.tensor.matmul(
                                ps[:, t * P:(t + 1) * P],
                                lhsT=wqk[:, eb, db * P:(db + 1) * P],
                                rhs=xc[:, t, eb, :],
                                start=(eb == 0), stop=(eb == EB - 1),
                            )
                    nc.vector.tensor_scalar_add(
                        qkT[b][:, db, sc * CH:(sc + 1) * CH], ps[:],
                        bqkf[:, db:db + 1])
                    yield
                for si in range(4):
                    sb = sc * 4 + si
                    pv_ = psm.tile([P, P], f32, tag="mid", name="psv")
                    for eb in range(EB):
                        nc.tensor.matmul(
                            pv_[:], lhsT=xc[:, si, eb, :],
                            rhs=wv[:, eb, :], start=(eb == 0), stop=False)
                    nc.tensor.matmul(pv_[:], lhsT=ones1[:1, :],
                                     rhs=auxr[:1, 0:P],
                                     start=False, stop=True)
                    nc.vector.tensor_copy(vsb[b][:, sb, 0, 0:64], pv_[:, 0:64])
                    nc.vector.tensor_copy(vsb[b][:, sb, 1, 0:64], pv_[:, 64:128])
                    yield

        def score_pieces(b, expst0, expst1):
            """scores^T + exp for BOTH heads of one batch, one k-block per
            piece. The heads' K=64 matmuls are emitted adjacently: head0 sits
            on partitions 0-63 and head1 on 64-127, so their inferred PE
            tile_positions are (0,0)/(64,0) and the array runs each pair
            nearly concurrently in its two row halves.

            k-blocks run high-to-low to match qkv_pieces' suffix-first order.
            """
            expst0.extend([None] * SBB)
            expst1.extend([None] * SBB)
            for kb in reversed(range(SBB)):
                L = S - kb * P
                # high-kb tiles are small: an extra buffer lets the next
                # batch's (reversed) scores start while this batch's PV is
                # still reading the lower k-blocks
                et = [epool.tile([P, L], bf16, tag=f"e{kb}", name=f"e{kb}",
                                 bufs=4 if kb >= 8 else 2)
                      for _ in range(2)]
                off = kb * P
                pos = 0
                while pos < L:  # 1024-wide psum tiles: 1 exp op per tile
                    c = min(1024, L - pos)
                    ps = [pbig.tile([P, 1024], f32, tag="big", name="pssc")
                          for _ in range(2)]
                    for c0 in range(0, c, 512):
                        w = min(512, c - c0)
                        for h in range(2):
                            hs = slice(h * 64, (h + 1) * 64)
                            nc.tensor.matmul(
                                ps[h][:, c0:c0 + w],
                                lhsT=qkT[b][hs, 1, off:off + P],
                                rhs=qkT[b][hs, 0,
                                           off + pos + c0:off + pos + c0 + w],
                                start=True, stop=True)
                    for h in range(2):
                        nc.scalar.activation(
                            et[h][:, pos:pos + c], ps[h][:, :c],
                            mybir.ActivationFunctionType.Exp)
                    pos += c
                # zero the invalid (q < k) half of the diagonal block
                for h in range(2):
                    nc.gpsimd.tensor_mul(et[h][:, 0:P], et[h][:, 0:P],
                                         aux[:, 2:2 + P])
                expst0[kb] = et[0]
                expst1[kb] = et[1]
                yield

        def pv_pieces(b, h, expst):
            """PV + normalize for one (batch, head), one q-tile per piece."""
            hs = slice(h * 64, (h + 1) * 64)
            for qt in range(SBB):
                pp = ppv.tile([P, 66], f32, tag="pv", name="pspv")
                for kb in range(qt + 1):
                    nc.tensor.matmul(
                        pp[:, 0:65],
                        lhsT=expst[kb][:, (qt - kb) * P:(qt - kb) * P + P],
                        rhs=vsb[b][:, kb, h, 0:65],
                        start=(kb == 0), stop=(kb == qt))
                rc = small.tile([P, 1], f32, tag="recip", name="rc")
                nc.vector.reciprocal(rc[:], pp[:, 64:65])
                nc.vector.tensor_scalar_mul(
                    attn[b][:, qt, hs], pp[:, 0:64], rc[:])
                yield

        def transpose_pieces(b):
            """attn [token, e] -> attnT [e, token] via PE transposes."""
            for blk in range(SBB):
                pt = psm.tile([P, P], bf16, tag="mid", name="pst")
                nc.tensor.transpose(pt[:], attn[b][:, blk, :], ident[:])
                nc.vector.tensor_copy(attnT[b][:, blk * P:(blk + 1) * P], pt[:])
                yield
            # two strided DMAs: chunk j of the bounce gets blocks {j, j+8}
            for t in range(2):
                nc.sync.dma_start(
                    a2a_in[b].ap().rearrange(
                        "j p (t c) -> p j t c", t=2)[:, :, t, :],
                    attnT[b][:, t * NCORES * P:(t + 1) * NCORES * P].rearrange(
                        "p (j c) -> p j c", c=P))
            yield

        def interleave(*gens):
            gens = list(gens)
            while gens:
                gens = [g for g in gens if next(g, StopIteration) is not StopIteration]

        def paced(qg, score_gens, pv_gens=()):
            """Weave one qkv stream with score/pv streams, pacing emission so
            every score k-block is emitted AFTER the qkv chunk that writes the
            qkT columns it reads (Tile only tracks writer->reader deps in
            emission order). qkv chunk g (suffix-first) unlocks score k-blocks
            [12-4g, 15-4g]."""
            for g in range(4):
                for _ in range(6):
                    next(qg, None)
                for _ in range(4):
                    for sg in score_gens:
                        next(sg, None)
                    for pg in pv_gens:
                        next(pg, None)
            interleave(qg, *score_gens, *pv_gens)

        atf = [work.tile([P, EB, 2 * P], bf16, tag=f"atf{b}", name=f"atf{b}")
               for b in range(B)]

        def a2a_batch(b):
            """AllToAll batch b: head-columns -> my two token blocks."""
            if no_cc:
                # cost-model variant: TimelineSim can't simulate collectives
                for j in range(NCORES):
                    nc.sync.dma_start(a2a_out[b][j], a2a_in[b][j])
            else:
                nc.gpsimd.collective_compute(
                    "AllToAll", mybir.AluOpType.bypass,
                    replica_groups=grp,
                    ins=[a2a_in[b].ap()], outs=[a2a_out[b].ap()])
            nc.sync.dma_start(
                atf[b][:, :, :],
                a2a_out[b].ap().rearrange("j p c -> p j c"))

        def oproj_batch(b):
            """Output projection of my two token blocks of batch b."""
            for st in range(2):
                ot = opool.tile([P, E], bf16, tag="o", name="ot")
                po = pbig.tile([P, 1024], f32, tag="big", name="pso")
                for oh in range(2):
                    for eb in range(EB):
                        nc.tensor.matmul(
                            po[:, oh * 512:(oh + 1) * 512],
                            lhsT=atf[b][:, eb, st * P:(st + 1) * P],
                            rhs=wo[:, eb, oh * 512:(oh + 1) * 512],
                            start=(eb == 0), stop=False)
                    nc.tensor.matmul(po[:, oh * 512:(oh + 1) * 512],
                                     lhsT=ones1[:1, :],
                                     rhs=auxr[:1, P + oh * 512:P + (oh + 1) * 512],
                                     start=False, stop=True)
                nc.vector.tensor_copy(ot[:], po[:])
                nc.sync.dma_start(out_d[b * 2 + st], ot[:])

        # ---- pipelined emission (priorities; Tile schedules by readiness) ----
        e00, e01, e10, e11 = [], [], [], []
        paced(qkv_pieces(0), [score_pieces(0, e00, e01)])
        paced(qkv_pieces(1),
              [score_pieces(1, e10, e11)],
              [pv_pieces(0, 0, e00), pv_pieces(0, 1, e01)])
        # full W_o materializes from the (long-finished) AllGather
        nc.sync.dma_start(wo[:], wag_out.ap().rearrange("(e p) n -> p e n", p=P))
        # batch-1 transposes ride along with batch-1 PV (piece qt is emitted
        # right after both heads' PV qt), so the final A2A fires early
        interleave(pv_pieces(1, 0, e10), pv_pieces(1, 1, e11),
                   transpose_pieces(0), transpose_pieces(1))
        a2a_batch(0)
        oproj_batch(0)          # hides under batch-1 attention tail
        a2a_batch(1)
        oproj_batch(1)

    nc.compile()
    return nc


def _make_runner(nc):
    """Build the shard_map'd PJRT callable ONCE; reuse across calls.

    Mirrors concourse.bass2jax.run_bass_via_pjrt, minus its per-call costs:
    the jit is cached (no retrace), inputs arrive as prebuilt global arrays
    (no per-call concat), and the donated output buffers are recycled from
    the previous call's device-side outputs (no 8.4MB of zeros on the wire;
    the kernel writes every output element so their content is irrelevant).
    """
    import jax
    import jax.numpy as jnp
    from jax.sharding import Mesh, NamedSharding, PartitionSpec
    from jax.experimental.shard_map import shard_map
    from concourse import bass2jax, mybir

    bass2jax.install_neuronx_cc_hook()
    assert nc.dbg_addr is None

    partition_name = nc.partition_id_tensor.name if nc.partition_id_tensor else None
    in_names, out_names, out_avals = [], [], []
    for alloc in nc.m.functions[0].allocations:
        if not isinstance(alloc, mybir.MemoryLocationSet):
            continue
        name = alloc.memorylocations[0].name
        if alloc.kind == "ExternalInput":
            if name != partition_name:
                in_names.append(name)
        elif alloc.kind == "ExternalOutput":
            out_names.append(name)
            shape = tuple(alloc.tensor_shape)
            dtype = mybir.dt.np(alloc.dtype)
            out_avals.append(jax.core.ShapedArray(shape, dtype))
    n_params = len(in_names)
    n_outs = len(out_avals)
    all_names = in_names + out_names + ([partition_name] if partition_name else [])
    donate = tuple(range(n_params, n_params + n_outs))

    def _body(*args):
        operands = list(args)
        if partition_name is not None:
            operands.append(bass2jax.partition_id_tensor())
        outs = bass2jax._bass_exec_p.bind(
            *operands,
            out_avals=tuple(out_avals),
            in_names=tuple(all_names),
            out_names=tuple(out_names),
            lowering_input_output_aliases=(),
            sim_require_finite=True,
            sim_require_nnan=True,
            nc=nc,
        )
        return tuple(outs)

    devices = jax.devices()[:NCORES]
    mesh = Mesh(np.asarray(devices), ("core",))
    spec = NamedSharding(mesh, PartitionSpec("core"))
    in_specs = (PartitionSpec("core"),) * (n_params + n_outs)
    out_specs = (PartitionSpec("core"),) * n_outs
    sharded = jax.jit(
        shard_map(_body, mesh=mesh, in_specs=in_specs, out_specs=out_specs,
                  check_rep=False),
        donate_argnums=donate, keep_unused=True)
    gshapes = [(NCORES * a.shape[0], *a.shape[1:]) for a in out_avals]
    mkzeros = jax.jit(
        lambda: tuple(jnp.zeros(s, a.dtype) for s, a in zip(gshapes, out_avals)),
        out_shardings=(spec,) * n_outs)
    state = {"don": None}

    def run(maps):
        gin = [maps[name] for name in in_names]
        don = state["don"]
        state["don"] = None
        if don is None:
            don = mkzeros()
        out_arrs = sharded(*gin, *don)
        res = [
            {name: np.asarray(out_arrs[i]).reshape(NCORES, *out_avals[i].shape)[c]
             for i, name in enumerate(out_names)}
            for c in range(NCORES)
        ]
        state["don"] = list(out_arrs)
        return res

    return run


def _in_maps(x, W_qkv, b_qkv, W_o, b_o):
    """Global [8*d0, ...] arrays (core c's shard = rows [c*d0, (c+1)*d0))."""
    # interleaved block sharding: core r's shard col k*128+c = x^T column of
    # global token block r+8k, so on-device AG chunk k = tokens [1024k, ...)
    xin = np.ascontiguousarray(
        x.reshape(NAG, NCORES, P, EB, P)
        .transpose(1, 4, 3, 0, 2)).astype(_bf16)  # [r, feat, eb, k, tok]
    winp = np.empty((NCORES * P, 4 * E), _bf16)
    aux = np.empty((NCORES * P, 2 + P), _bf16)
    tri = np.triu(np.ones((P, P), np.float32))
    auxr = np.empty((NCORES, P + E), _bf16)
    for c in range(NCORES):
        o = c * HPC * D
        q_sl = slice(o, o + HPC * D)
        k_sl = slice(E + o, E + o + HPC * D)
        v_sl = slice(2 * E + o, 2 * E + o + HPC * D)
        rows = slice(c * P, (c + 1) * P)
        wqk_c = np.concatenate(
            [W_qkv[:, q_sl] * 0.125, W_qkv[:, k_sl]], axis=1)
        winp[rows, 0:2 * E] = (
            wqk_c.reshape(EB, P, 2 * P).transpose(1, 0, 2).reshape(P, 2 * E))
        winp[rows, 2 * E:3 * E] = (
            W_qkv[:, v_sl].reshape(EB, P, P).transpose(1, 0, 2).reshape(P, E))
        winp[rows, 3 * E:] = W_o[c * P:(c + 1) * P, :]
        aux[rows, 0:2] = np.stack(
            [b_qkv[q_sl] * 0.125, b_qkv[k_sl]], axis=1)
        aux[rows, 2:] = tri
        auxr[c, 0:P] = b_qkv[v_sl]
        auxr[c, P:] = b_o
    return {
        "xin": xin.reshape(NCORES * P, EB, CH),
        "winp": winp,
        "aux": aux,
        "auxr": auxr,
    }


def _unshard(res):
    # rank r's out[st] is global 128-token block st*8 + r
    full = np.empty((SB, P, E), _bf16)
    for r in range(NCORES):
        full[r::NCORES] = res[r]["out"]
    return full.reshape(B, S, E).astype(np.float32)


def kernel(x, W_qkv, b_qkv, W_o, b_o, mask):
    if "nc" not in _cache:
        _cache["nc"] = _build()
    nc = _cache["nc"]
    maps = _in_maps(np.asarray(x, np.float32), np.asarray(W_qkv, np.float32),
                    np.asarray(b_qkv, np.float32), np.asarray(W_o, np.float32),
                    np.asarray(b_o, np.float32))
    try:
        if "runner" not in _cache:
            _cache["runner"] = _make_runner(nc)
        res = _cache["runner"](maps)
    except Exception:
        from concourse.bass_utils import run_bass_kernel_spmd
        per_core = [
            {k: v[c * (v.shape[0] // NCORES):(c + 1) * (v.shape[0] // NCORES)]
             for k, v in maps.items()}
            for c in range(NCORES)
        ]
        res = run_bass_kernel_spmd(nc, per_core, list(range(NCORES))).results
    return _unshard(res)
